# revision 1
# baseline (speedup 1.0000x reference)
"""DiffMHA (differential multi-head attention) block on 8 TRN2 NeuronCores.

Problem: B=4, L=1024, D=1024, H=16 heads (DH=64). Three input streams
(e_v, e_a0, e_a1); Q/K projections per stream, scores summed across
streams, causal-masked softmax, context from the v-stream values,
out-projection + residual + LayerNorm.

Sharding: (batch, head-half) -> 8 cores. Core c handles batch c//2 and
heads (c%2)*8 .. (c%2)*8+8. Each core computes its 8 heads' Q/K/V
projections (512 of 1024 channels), scores + softmax + context, and a
partial out-projection; a pairwise ReduceScatter sums the two partial
out-projections of a batch and splits rows, then each core applies
residual + LayerNorm on its 512 rows.

On-device layout notes:
- All activations feeding matmuls are kept CHANNEL-major ([c, l]
  "transposed" layout) so the PE contracts over partitions with zero
  on-device transposes; the host pre-transposes the embeddings.
- All matmul operands are bf16 (host-converted); PSUM accumulation is
  fp32 and the softmax/LN elementwise pipeline stays fp32.
- softmax runs in scores^T [k, q] layout: the per-q sum over k comes
  free from an extra ones-row appended to V (row 64 of each head's
  ctx PSUM accumulates sum_k attn), so no partition reductions and no
  attn transpose are needed.
- Fold-major schedule: as soon as channel-fold f (2 heads) of Q/K is
  projected for all three streams, those heads' attention runs and the
  fold tiles are recycled.
"""

import os
import sys
import types

import ml_dtypes
import numpy as np

B, L, D, H = 4, 1024, 1024, 16
DH = D // H
HPC = H // 2  # heads per core
C = HPC * DH  # channels per core (512)
SCALE = float(1.0 / np.sqrt(DH))
EPS = 1e-12
NCORES = 8
BF16 = ml_dtypes.bfloat16


def _install_ntff_hook():
    """Recreate antenv.axon_hooks (absent in this image) so
    run_bass_kernel_spmd(trace=True) can capture NTFF profiles."""
    if "antenv.axon_hooks" in sys.modules:
        return
    try:
        from trn_agent_boot.trn_boot import _ntff_profile_via_ctypes

        hook = _ntff_profile_via_ctypes("/opt/axon/libaxon_pjrt.so")
    except Exception:
        hook = None
    mod = types.ModuleType("antenv.axon_hooks")
    mod.get_axon_ntff_profile_hook = lambda: hook
    mod.set_axon_ntff_profile_hook = lambda h: None
    sys.modules["antenv.axon_hooks"] = mod


_install_ntff_hook()

import concourse.bass as bass  # noqa: E402
import concourse.mybir as mybir  # noqa: E402
import concourse.tile as tile  # noqa: E402
from concourse import bacc  # noqa: E402
from concourse.bass_utils import run_bass_kernel_spmd  # noqa: E402

F32 = mybir.dt.float32
BF = mybir.dt.bfloat16
AF = mybir.ActivationFunctionType
ALU = mybir.AluOpType

_NC_CACHE = {}
LAST_RESULT = None

NQF = C // 128  # 4 channel folds per stream (2 heads each)
NLT = L // 128  # 8 l-tiles
NDT = D // 128  # 8 d-tiles (contraction)
NKT = L // 128  # 8 k-tiles
NRF = (L // 2) // 128  # 4 row folds for LN
STREAMS = ("v", "a0", "a1")


def build_nc():
    nc = bacc.Bacc("TRN2", target_bir_lowering=False, debug=False, num_devices=NCORES)

    # ---- DRAM parameters (per-core shards, host-prepped) ----
    xt = {s: nc.declare_dram_parameter(f"xt_{s}", [D, L], BF, isOutput=False) for s in STREAMS}
    # W fold-sliced on host: [NQF, D, 128]
    wq = {s: nc.declare_dram_parameter(f"wq_{s}", [NQF, D, 128], BF, isOutput=False) for s in STREAMS}
    wk = {s: nc.declare_dram_parameter(f"wk_{s}", [NQF, D, 128], BF, isOutput=False) for s in STREAMS}
    wv = nc.declare_dram_parameter("wv", [D, C], BF, isOutput=False)
    wout = nc.declare_dram_parameter("wout", [C, D], BF, isOutput=False)
    bq = {s: nc.declare_dram_parameter(f"bq_{s}", [C], F32, isOutput=False) for s in STREAMS}
    bk = {s: nc.declare_dram_parameter(f"bk_{s}", [C], F32, isOutput=False) for s in STREAMS}
    bv = nc.declare_dram_parameter("bv", [1, C], BF, isOutput=False)
    bout_half = nc.declare_dram_parameter("bout_half", [1, D], BF, isOutput=False)
    maskt = nc.declare_dram_parameter("maskt", [L, L], BF, isOutput=False)
    ev_res = nc.declare_dram_parameter("ev_res", [L // 2, D], F32, isOutput=False)
    gamma = nc.declare_dram_parameter("gamma", [1, D], F32, isOutput=False)
    beta = nc.declare_dram_parameter("beta", [1, D], F32, isOutput=False)
    out = nc.declare_dram_parameter("out", [L // 2, D], F32, isOutput=True)

    with tile.TileContext(nc) as tc:
        with (
            tc.tile_pool(name="persist", bufs=1) as persist,
            tc.tile_pool(name="xtp", bufs=1) as xtp,
            tc.tile_pool(name="wf", bufs=8) as wf,
            tc.tile_pool(name="qkf", bufs=2) as qkf,
            tc.tile_pool(name="small", bufs=4) as small,
            tc.tile_pool(name="attn", bufs=4) as attn_pool,
            tc.tile_pool(name="ln", bufs=2) as ln_pool,
            tc.tile_pool(name="proj_ps", bufs=2, space="PSUM") as proj_ps,
            tc.tile_pool(name="sc_ps", bufs=2, space="PSUM") as sc_ps,
            tc.tile_pool(name="ctx_ps", bufs=2, space="PSUM") as ctx_ps,
            tc.tile_pool(name="dram", bufs=1, space="DRAM") as dram,
        ):
            # ---- persistent SBUF tensors ----
            vnat = persist.tile([128, NLT, HPC, DH + 1], BF, tag="vnat")
            ctxt = persist.tile([128, NQF, L], BF, tag="ctxt")
            maskt_sb = persist.tile([128, NKT, L], BF, tag="maskt")
            ones_b = persist.tile([1, L], BF, tag="ones")
            gb_bc = persist.tile([128, 2, D], F32, tag="gbbc")
            bout_sb = persist.tile([1, D], BF, tag="boutsb")
            bv_sb = persist.tile([1, C], BF, tag="bvsb")
            wout_sb = persist.tile([128, NQF, D], BF, tag="woutsb")
            eps_sb = persist.tile([128, 1], F32, tag="eps")
            bq_sb = {
                s: persist.tile([128, NQF], F32, tag=f"bq{s}", name=f"bq_sb_{s}")
                for s in STREAMS
            }
            bk_sb = {
                s: persist.tile([128, NQF], F32, tag=f"bk{s}", name=f"bk_sb_{s}")
                for s in STREAMS
            }

            nc.vector.memset(ones_b[:, :], 1.0)
            nc.vector.memset(eps_sb[:, :], EPS)
            # V ones-column (feeds the softmax-sum rows)
            nc.vector.memset(vnat[:, :, :, DH : DH + 1], 1.0)

            nc.sync.dma_start(
                out=maskt_sb[:, :, :],
                in_=maskt[:, :].rearrange("(kt p) q -> p kt q", p=128),
            )
            nc.sync.dma_start(out=bout_sb[:, :], in_=bout_half[:, :])
            nc.sync.dma_start(out=bv_sb[:, :], in_=bv[:, :])
            nc.sync.dma_start(
                out=wout_sb[:, :, :],
                in_=wout[:, :].rearrange("(cf p) d -> p cf d", p=128),
            )
            for s in STREAMS:
                nc.sync.dma_start(
                    out=bq_sb[s][:, :], in_=bq[s][:].rearrange("(f p) -> p f", p=128)
                )
                nc.sync.dma_start(
                    out=bk_sb[s][:, :], in_=bk[s][:].rearrange("(f p) -> p f", p=128)
                )
            gsb = small.tile([1, D], F32, tag="gsb", bufs=1)
            bsb = small.tile([1, D], F32, tag="bsb", bufs=1)
            nc.sync.dma_start(out=gsb[:, :], in_=gamma[:, :])
            nc.sync.dma_start(out=bsb[:, :], in_=beta[:, :])
            nc.gpsimd.partition_broadcast(gb_bc[:, 0, :], gsb[:, :])
            nc.gpsimd.partition_broadcast(gb_bc[:, 1, :], bsb[:, :])

            # ---- embeddings (channel-major), resident ----
            xt_sb = {}
            for s in STREAMS:
                t = xtp.tile([128, NDT, L], BF, tag=f"xt{s}", name=f"xt_sb_{s}")
                nc.sync.dma_start(
                    out=t[:, :, :],
                    in_=xt[s][:, :].rearrange("(dt p) l -> p dt l", p=128),
                )
                xt_sb[s] = t

            # ---- V projection (natural [l, c] layout + ones column) ----
            wv_sb = persist.tile([128, NDT, C], BF, tag="wvsb")
            nc.sync.dma_start(
                out=wv_sb[:, :, :],
                in_=wv[:, :].rearrange("(dt p) c -> p dt c", p=128),
            )
            for lf in range(NLT):
                ps = proj_ps.tile([128, C], F32, tag="proj")
                for dt in range(NDT):
                    nc.tensor.matmul(
                        ps[:, :],
                        xt_sb["v"][:, dt, lf * 128 : (lf + 1) * 128],
                        wv_sb[:, dt, :],
                        start=(dt == 0),
                        stop=False,
                    )
                # + bias via ones-row rank-1 update
                nc.tensor.matmul(
                    ps[:, :],
                    ones_b[:, lf * 128 : (lf + 1) * 128],
                    bv_sb[:, :],
                    start=False,
                    stop=True,
                )
                nc.scalar.copy(vnat[:, lf, :, 0:DH], ps[:, :])

            # ---- fold-major: project fold f for all streams, then run
            #      attention for heads 2f and 2f+1 ----
            for f in range(NQF):
                qtf, ktf = {}, {}
                for s in STREAMS:
                    wq_t = wf.tile([128, NDT, 128], BF, tag="w", name=f"wq_{s}{f}")
                    wk_t = wf.tile([128, NDT, 128], BF, tag="w", name=f"wk_{s}{f}")
                    nc.sync.dma_start(
                        out=wq_t[:, :, :],
                        in_=wq[s][f, :, :].rearrange("(dt p) c -> p dt c", p=128),
                    )
                    nc.sync.dma_start(
                        out=wk_t[:, :, :],
                        in_=wk[s][f, :, :].rearrange("(dt p) c -> p dt c", p=128),
                    )
                    for which, w_t, b_t, store in (
                        ("q", wq_t, bq_sb[s], qtf),
                        ("k", wk_t, bk_sb[s], ktf),
                    ):
                        ps = proj_ps.tile([128, L], F32, tag="proj")
                        for dt in range(NDT):
                            for lh in range(2):
                                nc.tensor.matmul(
                                    ps[:, lh * 512 : (lh + 1) * 512],
                                    w_t[:, dt, :],
                                    xt_sb[s][:, dt, lh * 512 : (lh + 1) * 512],
                                    start=(dt == 0),
                                    stop=(dt == NDT - 1),
                                )
                        dst = qkf.tile(
                            [128, L], BF, tag=f"{which}t{s}", name=f"{which}t_{s}{f}"
                        )
                        nc.scalar.activation(
                            dst[:, :], ps[:, :], AF.Identity, bias=b_t[:, f : f + 1]
                        )
                        store[s] = dst

                for hh in range(2):  # head within fold
                    h = 2 * f + hh
                    p0 = hh * 64
                    for qh in range(2):
                        qsl = slice(qh * 512, (qh + 1) * 512)
                        cps = ctx_ps.tile([DH + 1, 512], F32, tag="ctx")
                        for kt_i in range(NKT):
                            ksl = slice(kt_i * 128, (kt_i + 1) * 128)
                            sps = sc_ps.tile([128, 512], F32, tag="sc")
                            for i, s in enumerate(STREAMS):
                                nc.tensor.matmul(
                                    sps[:, :],
                                    ktf[s][p0 : p0 + 64, ksl],
                                    qtf[s][p0 : p0 + 64, qsl],
                                    start=(i == 0),
                                    stop=(i == 2),
                                )
                            # add (mask / SCALE), then exp(SCALE * x)
                            nc.vector.tensor_add(
                                sps[:, :], sps[:, :], maskt_sb[:, kt_i, qsl]
                            )
                            attn_sb = attn_pool.tile([128, 512], BF, tag="attn")
                            nc.scalar.activation(
                                attn_sb[:, :], sps[:, :], AF.Exp, scale=SCALE
                            )
                            nc.tensor.matmul(
                                cps[:, :],
                                vnat[:, kt_i, h, :],
                                attn_sb[:, :],
                                start=(kt_i == 0),
                                stop=(kt_i == NKT - 1),
                            )
                        inv = small.tile([1, 512], F32, tag="inv", bufs=2)
                        nc.vector.reciprocal(inv[:, :], cps[DH : DH + 1, :])
                        inv_bc = small.tile([64, 512], F32, tag="invbc", bufs=2)
                        nc.gpsimd.partition_broadcast(inv_bc[:, :], inv[:, :])
                        nc.vector.tensor_mul(
                            ctxt[p0 : p0 + 64, f, qsl], cps[0:DH, :], inv_bc[:, :]
                        )

            # ---- partial out-projection ----
            partial = dram.tile([L, D], F32)
            for lt in range(NLT):
                lsl = slice(lt * 128, (lt + 1) * 128)
                ops = proj_ps.tile([128, D], F32, tag="proj")
                for dh_i in range(2):
                    dsl = slice(dh_i * 512, (dh_i + 1) * 512)
                    for cf in range(NQF):
                        nc.tensor.matmul(
                            ops[:, dsl],
                            ctxt[:, cf, lsl],
                            wout_sb[:, cf, dsl],
                            start=(cf == 0),
                            stop=False,
                        )
                    nc.tensor.matmul(
                        ops[:, dsl],
                        ones_b[:, lsl],
                        bout_sb[:, dsl],
                        start=False,
                        stop=True,
                    )
                op_sb = ln_pool.tile([128, D], F32, tag="x")
                nc.scalar.copy(op_sb[:, :], ops[:, :])
                nc.sync.dma_start(out=partial[lsl, :], in_=op_sb[:, :])

            # ---- ReduceScatter over batch pairs ----
            rs_out = dram.tile([L // 2, D], F32)
            nc.gpsimd.collective_compute(
                "ReduceScatter",
                ALU.add,
                replica_groups=[[0, 1], [2, 3], [4, 5], [6, 7]],
                ins=[partial.opt()],
                outs=[rs_out.opt()],
            )

            # ---- residual + LayerNorm on own 512 rows ----
            for rf in range(NRF):
                rsl = slice(rf * 128, (rf + 1) * 128)
                x_sb = ln_pool.tile([128, D], F32, tag="x")
                nc.sync.dma_start(out=x_sb[:, :], in_=rs_out[rsl, :])
                ev_sb = ln_pool.tile([128, D], F32, tag="ev")
                nc.sync.dma_start(out=ev_sb[:, :], in_=ev_res[rsl, :])
                nc.vector.tensor_add(x_sb[:, :], x_sb[:, :], ev_sb[:, :])
                stats = small.tile([128, 2, 6], F32, tag="stats")
                nc.vector.bn_stats(out=stats[:, 0, :], in_=x_sb[:, 0:512])
                nc.vector.bn_stats(out=stats[:, 1, :], in_=x_sb[:, 512:1024])
                mv = small.tile([128, 2], F32, tag="mv")
                nc.vector.bn_aggr(out=mv[:, :], in_=stats[:, :, :])
                std = small.tile([128, 1], F32, tag="std")
                nc.scalar.activation(std[:, :], mv[:, 1:2], AF.Sqrt, bias=eps_sb[:, :])
                rstd = small.tile([128, 1], F32, tag="rstd")
                nc.vector.reciprocal(rstd[:, :], std[:, :])
                negmb = small.tile([128, 1], F32, tag="negmb")
                nc.vector.scalar_tensor_tensor(
                    negmb[:, :],
                    mv[:, 0:1],
                    -1.0,
                    rstd[:, :],
                    op0=ALU.mult,
                    op1=ALU.mult,
                )
                nc.scalar.activation(
                    x_sb[:, :],
                    x_sb[:, :],
                    AF.Identity,
                    bias=negmb[:, :],
                    scale=rstd[:, :],
                )
                nc.vector.tensor_mul(x_sb[:, :], x_sb[:, :], gb_bc[:, 0, :])
                nc.vector.tensor_add(x_sb[:, :], x_sb[:, :], gb_bc[:, 1, :])
                nc.sync.dma_start(out=out[rsl, :], in_=x_sb[:, :])

    nc.compile()
    return nc


def _get_nc():
    if "nc" not in _NC_CACHE:
        _NC_CACHE["nc"] = build_nc()
    return _NC_CACHE["nc"]


def kernel(
    e_v, e_a0, e_a1, Wqv, bqv, Wkv, bkv, Wvv, bvv,
    Wqa0, bqa0, Wka0, bka0, Wqa1, bqa1, Wka1, bka1,
    Wout, bout, ln_gamma, ln_beta, attn_mask,
):
    global LAST_RESULT
    f = np.asarray
    e_v, e_a0, e_a1 = f(e_v), f(e_a0), f(e_a1)
    attn_mask = f(attn_mask)
    c32 = lambda a: np.ascontiguousarray(a, dtype=np.float32)
    cbf = lambda a: np.ascontiguousarray(np.asarray(a, dtype=np.float32).astype(BF16))

    wq_full = {"v": f(Wqv), "a0": f(Wqa0), "a1": f(Wqa1)}
    wk_full = {"v": f(Wkv), "a0": f(Wka0), "a1": f(Wka1)}
    bq_full = {"v": f(bqv), "a0": f(bqa0), "a1": f(bqa1)}
    bk_full = {"v": f(bkv), "a0": f(bka0), "a1": f(bka1)}

    xts = {}
    maskts = {}
    for b in range(B):
        xts[b] = {
            "v": cbf(e_v[b].T),
            "a0": cbf(e_a0[b].T),
            "a1": cbf(e_a1[b].T),
        }
        maskts[b] = cbf(attn_mask[b, 0].T * (1.0 / SCALE))

    def fold_slice(w, S):
        # [D, C] slice -> [NQF, D, 128] fold-major
        ws = np.asarray(w[:, S], dtype=np.float32)  # [D, C]
        return np.ascontiguousarray(
            ws.reshape(D, NQF, 128).transpose(1, 0, 2).astype(BF16)
        )

    in_maps = []
    for c in range(NCORES):
        b, hh = c // 2, c % 2
        S = slice(hh * C, (hh + 1) * C)
        m = {}
        for s in STREAMS:
            m[f"xt_{s}"] = xts[b][s]
            m[f"wq_{s}"] = fold_slice(wq_full[s], S)
            m[f"wk_{s}"] = fold_slice(wk_full[s], S)
            m[f"bq_{s}"] = c32(bq_full[s][S])
            m[f"bk_{s}"] = c32(bk_full[s][S])
        m["wv"] = cbf(f(Wvv)[:, S])
        m["bv"] = cbf(f(bvv)[S]).reshape(1, C)
        m["wout"] = cbf(f(Wout)[S, :])
        m["bout_half"] = cbf(f(bout) * 0.5).reshape(1, D)
        m["maskt"] = maskts[b]
        m["ev_res"] = c32(e_v[b, hh * 512 : (hh + 1) * 512, :])
        m["gamma"] = c32(f(ln_gamma)).reshape(1, D)
        m["beta"] = c32(f(ln_beta)).reshape(1, D)
        in_maps.append(m)

    nc = _get_nc()
    trace = bool(os.environ.get("KERNEL_TRACE"))
    res = run_bass_kernel_spmd(
        nc, in_maps, core_ids=list(range(NCORES)), trace=trace
    )
    LAST_RESULT = res

    out_full = np.empty((B, L, D), dtype=np.float32)
    for c in range(NCORES):
        b, hh = c // 2, c % 2
        out_full[b, hh * 512 : (hh + 1) * 512, :] = res.results[c]["out"]
    return out_full



# revision 19
# speedup vs baseline: 1.8459x; 1.8459x over previous
"""DiffMHA (differential multi-head attention) block on 8 TRN2 NeuronCores.

Problem: B=4, L=1024, D=1024, H=16 heads (DH=64). Three input streams
(e_v, e_a0, e_a1); Q/K projections per stream, scores summed across
streams, causal-masked softmax, context from the v-stream values,
out-projection + residual + LayerNorm.

Sharding: (batch, head-half) -> 8 cores. Core c handles batch c//2 and
heads (c%2)*8 .. (c%2)*8+8. Each core computes its 8 heads' Q/K/V
projections (512 of 1024 channels), scores + softmax + context. Fold
context halves (128 channels x 512 rows) are exchanged between the two
cores of a batch via per-fold pairwise AllToAll DURING attention; each
core then runs the out-projection for its own 512 sequence rows with
the full 1024-channel contraction, then residual + LayerNorm locally.
No end-of-kernel collective.

Key optimizations over the v1 kernel:
- Causal skipping: score tiles with kt*128 > qb*256+255 are fully
  masked and skipped entirely (exp == 0 exactly); only diagonal-band
  tiles get the mask add. Attention matmul work drops ~40%.
- Stream packing: q/k of streams v and a0 are packed onto 128
  partitions (64 chans each) at projection-eviction time (partition-
  shifted PSUM->SBUF copies), so their two 64-deep score matmuls fuse
  into one 128-deep matmul; stream a1 stays a 64-deep accumulate.
- PSUM bank alternation: consecutive PE matmuls never accumulate into
  the same PSUM bank back-to-back (V-proj lf pairs, per-head score
  tiles, ctx of 2 heads, out-proj chains are interleaved), which keeps
  the PE at its ~216ns/512-col pipelined rate instead of ~430ns.
- Softmax normalization deferred past ctx accumulation via the extra
  ones-row of V (unchanged), reciprocal+broadcast per (head, q-half).
- DMA order: xt_v + wv first so the PE starts ~10us in, not ~46us.
"""

import os
import sys
import types

import ml_dtypes
import numpy as np

B, L, D, H = 4, 1024, 1024, 16
DH = D // H
HPC = H // 2  # heads per core
C = HPC * DH  # channels per core (512)
SCALE = float(1.0 / np.sqrt(DH))
EPS = 1e-12
NCORES = 8
BF16 = ml_dtypes.bfloat16


def _install_ntff_hook():
    """Recreate antenv.axon_hooks (absent in this image) so
    run_bass_kernel_spmd(trace=True) can capture NTFF profiles."""
    if "antenv.axon_hooks" in sys.modules:
        return
    try:
        from trn_agent_boot.trn_boot import _ntff_profile_via_ctypes

        hook = _ntff_profile_via_ctypes("/opt/axon/libaxon_pjrt.so")
    except Exception:
        hook = None
    mod = types.ModuleType("antenv.axon_hooks")
    mod.get_axon_ntff_profile_hook = lambda: hook
    mod.set_axon_ntff_profile_hook = lambda h: None
    sys.modules["antenv.axon_hooks"] = mod


_install_ntff_hook()

import concourse.bass as bass  # noqa: E402
import concourse.mybir as mybir  # noqa: E402
import concourse.tile as tile  # noqa: E402
from concourse import bacc  # noqa: E402
from concourse.bass_utils import run_bass_kernel_spmd  # noqa: E402

F32 = mybir.dt.float32
BF = mybir.dt.bfloat16
AF = mybir.ActivationFunctionType
ALU = mybir.AluOpType

_NC_CACHE = {}
LAST_RESULT = None

NQF = C // 128  # 4 channel folds per stream (2 heads each)
NLT = L // 128  # 8 l-tiles
NDT = D // 128  # 8 d-tiles (contraction)
NKT = L // 128  # 8 k-tiles
NRF = (L // 2) // 128  # 4 row tiles for out-proj/LN
STREAMS = ("v", "a0", "a1")
GROUPS = [[0, 1], [2, 3], [4, 5], [6, 7]]


def build_nc():
    nc = bacc.Bacc("TRN2", target_bir_lowering=False, debug=False, num_devices=NCORES)

    # ---- DRAM parameters (per-core shards, host-prepped) ----
    xt = {s: nc.declare_dram_parameter(f"xt_{s}", [D, L], BF, isOutput=False) for s in STREAMS}
    wq = {s: nc.declare_dram_parameter(f"wq_{s}", [NQF, D, 128], BF, isOutput=False) for s in STREAMS}
    wk = {s: nc.declare_dram_parameter(f"wk_{s}", [NQF, D, 128], BF, isOutput=False) for s in STREAMS}
    wv = nc.declare_dram_parameter("wv", [D, C], BF, isOutput=False)
    wout8 = nc.declare_dram_parameter("wout8", [128, 8, D], BF, isOutput=False)
    bq = {s: nc.declare_dram_parameter(f"bq_{s}", [C], F32, isOutput=False) for s in STREAMS}
    bk = {s: nc.declare_dram_parameter(f"bk_{s}", [C], F32, isOutput=False) for s in STREAMS}
    bv = nc.declare_dram_parameter("bv", [1, C], BF, isOutput=False)
    maskAB = nc.declare_dram_parameter("maskAB", [128, 2, 256], BF, isOutput=False)
    sel = nc.declare_dram_parameter("sel", [128, 2, 512], BF, isOutput=False)
    ev_res = nc.declare_dram_parameter("ev_res", [L // 2, D], F32, isOutput=False)
    gamma = nc.declare_dram_parameter("gamma", [1, D], F32, isOutput=False)
    beta = nc.declare_dram_parameter("beta", [1, D], F32, isOutput=False)
    out = nc.declare_dram_parameter("out", [L // 2, D], F32, isOutput=True)


    with tile.TileContext(nc) as tc:
        with (
            tc.tile_pool(name="persist", bufs=1) as persist,
            tc.tile_pool(name="xtp", bufs=1) as xtp,
            tc.tile_pool(name="wf", bufs=10) as wf,
            tc.tile_pool(name="qkf", bufs=2) as qkf,
            tc.tile_pool(name="small", bufs=4) as small,
            tc.tile_pool(name="attn", bufs=4) as attn_pool,
            tc.tile_pool(name="ln", bufs=2) as ln_pool,
            tc.tile_pool(name="evp", bufs=4) as evp,
            tc.tile_pool(name="ctxf", bufs=2) as ctxf_pool,
            tc.tile_pool(name="proj_ps", bufs=3, space="PSUM") as proj_ps,
            tc.tile_pool(name="sc_ps", bufs=3, space="PSUM") as sc_ps,
            tc.tile_pool(name="ctx_ps", bufs=2, space="PSUM") as ctx_ps,
            tc.tile_pool(name="dram", bufs=1, space="DRAM") as dram,
        ):
            # ---- persistent SBUF tensors ----
            vnat = persist.tile([128, NLT, HPC, DH + 1], BF, tag="vnat")
            ctx_all = persist.tile([128, 8, L // 2], BF, tag="ctxall")
            mask_sb = persist.tile([128, 2, 256], BF, tag="maskAB")
            ones_b = persist.tile([1, L], BF, tag="ones")
            gb_bc = persist.tile([128, 2, D], F32, tag="gbbc")
            bv_sb = persist.tile([1, C], BF, tag="bvsb")
            wv_sb = persist.tile([128, NDT, C], BF, tag="wvsb")
            wout_sb = persist.tile([128, 8, D], BF, tag="woutsb")
            eps_sb = persist.tile([128, 1], F32, tag="eps")
            bq_sb = {
                s: persist.tile([128, NQF], F32, tag=f"bq{s}", name=f"bq_sb_{s}")
                for s in STREAMS
            }
            bk_sb = {
                s: persist.tile([128, NQF], F32, tag=f"bk{s}", name=f"bk_sb_{s}")
                for s in STREAMS
            }

            # ---- fold weight loader (lazy, cached) ----
            wf_cache = {}

            def load_wf(s, ff):
                if (s, ff) in wf_cache:
                    return wf_cache[(s, ff)]
                wq_t = wf.tile([128, NDT, 128], BF, tag="w", name=f"wq_{s}{ff}")
                wk_t = wf.tile([128, NDT, 128], BF, tag="w", name=f"wk_{s}{ff}")
                nc.sync.dma_start(
                    out=wq_t[:, :, :],
                    in_=wq[s][ff, :, :].rearrange("(dt p) c -> p dt c", p=128),
                )
                nc.sync.dma_start(
                    out=wk_t[:, :, :],
                    in_=wk[s][ff, :, :].rearrange("(dt p) c -> p dt c", p=128),
                )
                wf_cache[(s, ff)] = (wq_t, wk_t)
                return wq_t, wk_t

            # ---- preamble DMAs, in critical-path order: V-proj deps
            #      first, then fold-0 Q/K weights interleaved with the
            #      remaining embeddings; big late-use tensors (wout, ev,
            #      gamma/beta) are deferred into the fold loop. ----
            xt_sb = {}
            t = xtp.tile([128, NDT, L], BF, tag="xtv", name="xt_sb_v")
            nc.sync.dma_start(
                out=t[:, :, :], in_=xt["v"][:, :].rearrange("(dt p) l -> p dt l", p=128)
            )
            xt_sb["v"] = t
            nc.sync.dma_start(
                out=wv_sb[:, :, :], in_=wv[:, :].rearrange("(dt p) c -> p dt c", p=128)
            )
            nc.sync.dma_start(out=bv_sb[:, :], in_=bv[:, :])
            load_wf("v", 0)
            for s in ("a0", "a1"):
                t = xtp.tile([128, NDT, L], BF, tag=f"xt{s}", name=f"xt_sb_{s}")
                nc.sync.dma_start(
                    out=t[:, :, :],
                    in_=xt[s][:, :].rearrange("(dt p) l -> p dt l", p=128),
                )
                xt_sb[s] = t
                load_wf(s, 0)

            nc.vector.memset(ones_b[:, :], 1.0)
            nc.vector.memset(eps_sb[:, :], EPS)
            nc.vector.memset(vnat[:, :, :, DH : DH + 1], 1.0)

            nc.sync.dma_start(out=mask_sb[:, :, :], in_=maskAB[:, :, :])
            sel_sb = persist.tile([128, 2, 512], BF, tag="sel")
            nc.sync.dma_start(out=sel_sb[:, :, :], in_=sel[:, :, :])
            for s in STREAMS:
                nc.sync.dma_start(
                    out=bq_sb[s][:, :], in_=bq[s][:].rearrange("(f p) -> p f", p=128)
                )
                nc.sync.dma_start(
                    out=bk_sb[s][:, :], in_=bk[s][:].rearrange("(f p) -> p f", p=128)
                )
            ev_sb = []

            # ---- V projection: natural [l, c] layout + ones column.
            #      lf pairs interleaved so consecutive matmuls alternate
            #      PSUM banks. ----
            for pair in range(NLT // 2):
                lf0, lf1 = 2 * pair, 2 * pair + 1
                psA = sc_ps.tile([128, C], F32, tag="sc")
                psB = sc_ps.tile([128, C], F32, tag="sc")
                for dt in range(NDT):
                    for lf, ps in ((lf0, psA), (lf1, psB)):
                        nc.tensor.matmul(
                            ps[:, :],
                            xt_sb["v"][:, dt, lf * 128 : (lf + 1) * 128],
                            wv_sb[:, dt, :],
                            start=(dt == 0),
                            stop=False,
                        )
                for lf, ps in ((lf0, psA), (lf1, psB)):
                    nc.tensor.matmul(
                        ps[:, :],
                        ones_b[:, lf * 128 : (lf + 1) * 128],
                        bv_sb[:, :],
                        start=False,
                        stop=True,
                    )
                nc.scalar.copy(vnat[:, lf0, :, 0:DH], psA[:, :])
                nc.scalar.copy(vnat[:, lf1, :, 0:DH], psB[:, :])

            # ---- fold-major main loop ----
            for f in range(NQF):
                # -- Q/K projections for all three streams --
                # packed tiles: partitions [0:64] = stream v chans of the
                # head, [64:128] = stream a0 chans; a1 keeps fold layout.
                qpk = [
                    qkf.tile([128, L], BF, tag=f"qpk{hh}", name=f"qpk{hh}_{f}")
                    for hh in range(2)
                ]
                kpk = [
                    qkf.tile([128, L], BF, tag=f"kpk{hh}", name=f"kpk{hh}_{f}")
                    for hh in range(2)
                ]
                qa1 = qkf.tile([128, L], BF, tag="qa1", name=f"qa1_{f}")
                ka1 = qkf.tile([128, L], BF, tag="ka1", name=f"ka1_{f}")

                # stage late-use loads into fold slots so they don't
                # compete with the critical-path preamble DMAs
                if f == 1:
                    nc.sync.dma_start(out=wout_sb[:, :, :], in_=wout8[:, :, :])
                if f == 2:
                    for rf in range(NRF):
                        t = evp.tile([128, D], F32, tag="ev", name=f"ev{rf}")
                        nc.sync.dma_start(
                            out=t[:, :], in_=ev_res[rf * 128 : (rf + 1) * 128, :]
                        )
                        ev_sb.append(t)
                if f == 3:
                    gsb = small.tile([1, D], F32, tag="gsb", bufs=1)
                    bsb = small.tile([1, D], F32, tag="bsb", bufs=1)
                    nc.sync.dma_start(out=gsb[:, :], in_=gamma[:, :])
                    nc.sync.dma_start(out=bsb[:, :], in_=beta[:, :])
                    nc.gpsimd.partition_broadcast(gb_bc[:, 0, :], gsb[:, :])
                    nc.gpsimd.partition_broadcast(gb_bc[:, 1, :], bsb[:, :])

                for s in STREAMS:
                    wq_t, wk_t = load_wf(s, f)
                    if f + 1 < NQF:
                        load_wf(s, f + 1)  # prefetch next fold's weights
                    for w_t, b_t, pk, a1t in (
                        (wq_t, bq_sb[s], qpk, qa1),
                        (wk_t, bk_sb[s], kpk, ka1),
                    ):
                        ps = [
                            proj_ps.tile([128, 512], F32, tag="proj", name=f"pp{lh}")
                            for lh in range(2)
                        ]
                        for dt in range(NDT):
                            for lh in range(2):
                                nc.tensor.matmul(
                                    ps[lh][:, :],
                                    w_t[:, dt, :],
                                    xt_sb[s][:, dt, lh * 512 : (lh + 1) * 512],
                                    start=(dt == 0),
                                    stop=(dt == NDT - 1),
                                )
                        for lh in range(2):
                            lsl = slice(lh * 512, (lh + 1) * 512)
                            if s == "a1":
                                nc.scalar.activation(
                                    a1t[:, lsl], ps[lh][:, :], AF.Identity,
                                    bias=b_t[:, f : f + 1],
                                )
                            else:
                                off = 0 if s == "v" else 64
                                for hh in range(2):
                                    nc.scalar.activation(
                                        pk[hh][off : off + 64, lsl],
                                        ps[lh][hh * 64 : hh * 64 + 64, :],
                                        AF.Identity,
                                        bias=b_t[hh * 64 : hh * 64 + 64, f : f + 1],
                                    )

                # -- attention for heads 2f (A: parts 0:64) and 2f+1 (B) --
                ctxf = ctxf_pool.tile([128, L], BF, tag="ctxf", name=f"ctxf{f}")
                for qh in range(2):
                    cps = [
                        ctx_ps.tile([DH + 1, 512], F32, tag="ctx", name=f"cps{i}")
                        for i in range(2)
                    ]
                    n_kt = 4 * qh + 4  # live k-tiles for this q-half
                    last_live = (4 * qh + 1, 4 * qh + 3)  # per qb half
                    sps_at = {}

                    def emit_scores(kt):
                        sps = [
                            sc_ps.tile([128, 512], F32, tag="sc", name=f"sps{i}")
                            for i in range(2)
                        ]
                        halves = [0, 1] if kt <= 4 * qh + 1 else [1]
                        for qbh in halves:
                            qsl = slice(qh * 512 + qbh * 256, qh * 512 + qbh * 256 + 256)
                            osl = slice(qbh * 256, qbh * 256 + 256)
                            ksl = slice(kt * 128, (kt + 1) * 128)
                            for hh in range(2):
                                nc.tensor.matmul(
                                    sps[hh][:, osl],
                                    kpk[hh][:, ksl],
                                    qpk[hh][:, qsl],
                                    start=True,
                                    stop=False,
                                )
                            for hh in range(2):
                                p0 = hh * 64
                                nc.tensor.matmul(
                                    sps[hh][:, osl],
                                    ka1[p0 : p0 + 64, ksl],
                                    qa1[p0 : p0 + 64, qsl],
                                    start=False,
                                    stop=True,
                                )
                        # mask only on diagonal-band halves
                        for qbh in halves:
                            qb = 2 * qh + qbh
                            if kt in (2 * qb, 2 * qb + 1):
                                osl = slice(qbh * 256, qbh * 256 + 256)
                                for hh in range(2):
                                    nc.vector.tensor_add(
                                        sps[hh][:, osl],
                                        sps[hh][:, osl],
                                        mask_sb[:, kt % 2, :],
                                    )
                        # exp -> bf16 attn tiles (dead qb0 half zeroed so
                        # the full-width ctx matmul accumulates one group
                        # per PSUM bank)
                        at = [
                            attn_pool.tile([128, 512], BF, tag="attn", name=f"at{i}")
                            for i in range(2)
                        ]
                        full = kt <= 4 * qh + 1
                        esl = slice(0, 512) if full else slice(256, 512)
                        for hh in range(2):
                            if not full:
                                nc.vector.memset(at[hh][:, 0:256], 0.0)
                            nc.scalar.activation(
                                at[hh][:, esl], sps[hh][:, esl], AF.Exp, scale=SCALE
                            )
                        sps_at[kt] = at

                    def emit_ctx(kt):
                        at = sps_at.pop(kt)
                        for hh in range(2):
                            h = 2 * f + hh
                            nc.tensor.matmul(
                                cps[hh][:, :],
                                vnat[:, kt, h, :],
                                at[hh][:, :],
                                start=(kt == 0),
                                stop=(kt == n_kt - 1),
                            )

                    prev = None
                    for kt in range(n_kt):
                        emit_scores(kt)
                        if prev is not None:
                            emit_ctx(prev)
                        prev = kt
                    emit_ctx(prev)

                    # normalize: divide ctx rows by the attn row-sums that
                    # accumulated in psum row DH
                    for hh in range(2):
                        p0 = hh * 64
                        inv = small.tile([1, 512], F32, tag="inv", bufs=2)
                        nc.vector.reciprocal(inv[:, :], cps[hh][DH : DH + 1, :])
                        inv_bc = small.tile([64, 512], F32, tag="invbc", bufs=2)
                        nc.gpsimd.partition_broadcast(inv_bc[:, :], inv[:, :])
                        nc.vector.tensor_mul(
                            ctxf[p0 : p0 + 64, qh * 512 : (qh + 1) * 512],
                            cps[hh][0:DH, :],
                            inv_bc[:, :],
                        )

                # -- exchange fold ctx halves with the pair core.
                # AllToAll isn't available for 2-core groups, so emulate it
                # with a ReduceScatter over [dest d][chan-slot s] staging
                # where slot s != own-half is zeroed via the host-provided
                # 0/1 `sel` tensor (x + 0 is exact in bf16). Rank d then
                # receives [ctx_half0, ctx_half1] for its own rows. --
                cxs = ctxf_pool.tile([128, 2, 2, 512], BF, tag="cxs", name=f"cxs{f}")
                for d2 in range(2):
                    for s2 in range(2):
                        nc.vector.tensor_mul(
                            cxs[:, d2, s2, :],
                            ctxf[:, d2 * 512 : (d2 + 1) * 512],
                            sel_sb[:, s2, :],
                        )
                cx_in = dram.tile(
                    [2, 2, 128, 512], BF, tag=f"cxin{f}", name=f"cxin{f}"
                )
                cx_out = dram.tile(
                    [2, 128, 512], BF, tag=f"cxout{f}", name=f"cxout{f}"
                )
                for d2 in range(2):
                    for s2 in range(2):
                        nc.sync.dma_start(
                            out=cx_in[d2, s2, :, :], in_=cxs[:, d2, s2, :]
                        )
                nc.gpsimd.collective_compute(
                    "ReduceScatter",
                    ALU.add,
                    replica_groups=GROUPS,
                    ins=[cx_in.opt()],
                    outs=[cx_out.opt()],
                )
                for s2 in range(2):
                    nc.sync.dma_start(
                        out=ctx_all[:, s2 * NQF + f, :], in_=cx_out[s2, :, :]
                    )

            # ---- out-projection over full 1024 channels for own rows ----
            # 8 chains (lt, dh); consecutive chains interleaved per cf so
            # PSUM banks alternate.
            x_sb = [None] * NRF
            for cp in range(NRF):  # chain pair = one lt row tile
                lt = cp
                lsl = slice(lt * 128, (lt + 1) * 128)
                ops = [
                    sc_ps.tile([128, 512], F32, tag="sc", name=f"ops{i}")
                    for i in range(2)
                ]
                for cf in range(8):
                    for dh in range(2):
                        nc.tensor.matmul(
                            ops[dh][:, :],
                            ctx_all[:, cf, lsl],
                            wout_sb[:, cf, dh * 512 : (dh + 1) * 512],
                            start=(cf == 0),
                            stop=(cf == 7),
                        )
                xt_ = ln_pool.tile([128, D], F32, tag="x", name=f"x{lt}")
                nc.scalar.copy(xt_[:, 0:512], ops[0][:, :])
                nc.scalar.copy(xt_[:, 512:1024], ops[1][:, :])
                x_sb[lt] = xt_

                # ---- residual + LayerNorm on this row tile ----
                nc.vector.tensor_add(xt_[:, :], xt_[:, :], ev_sb[lt][:, :])
                stats = small.tile([128, 2, 6], F32, tag="stats")
                nc.vector.bn_stats(out=stats[:, 0, :], in_=xt_[:, 0:512])
                nc.vector.bn_stats(out=stats[:, 1, :], in_=xt_[:, 512:1024])
                mv = small.tile([128, 2], F32, tag="mv")
                nc.vector.bn_aggr(out=mv[:, :], in_=stats[:, :, :])
                std = small.tile([128, 1], F32, tag="std")
                nc.scalar.activation(std[:, :], mv[:, 1:2], AF.Sqrt, bias=eps_sb[:, :])
                rstd = small.tile([128, 1], F32, tag="rstd")
                nc.vector.reciprocal(rstd[:, :], std[:, :])
                negmb = small.tile([128, 1], F32, tag="negmb")
                nc.vector.scalar_tensor_tensor(
                    negmb[:, :],
                    mv[:, 0:1],
                    -1.0,
                    rstd[:, :],
                    op0=ALU.mult,
                    op1=ALU.mult,
                )
                nc.scalar.activation(
                    xt_[:, :],
                    xt_[:, :],
                    AF.Identity,
                    bias=negmb[:, :],
                    scale=rstd[:, :],
                )
                nc.vector.tensor_mul(xt_[:, :], xt_[:, :], gb_bc[:, 0, :])
                nc.vector.tensor_add(xt_[:, :], xt_[:, :], gb_bc[:, 1, :])
                nc.sync.dma_start(out=out[lsl, :], in_=xt_[:, :])

    nc.compile()
    return nc


def _get_nc():
    if "nc" not in _NC_CACHE:
        _NC_CACHE["nc"] = build_nc()
    return _NC_CACHE["nc"]


def kernel(
    e_v, e_a0, e_a1, Wqv, bqv, Wkv, bkv, Wvv, bvv,
    Wqa0, bqa0, Wka0, bka0, Wqa1, bqa1, Wka1, bka1,
    Wout, bout, ln_gamma, ln_beta, attn_mask,
):
    global LAST_RESULT
    f = np.asarray
    e_v, e_a0, e_a1 = f(e_v), f(e_a0), f(e_a1)
    attn_mask = f(attn_mask)
    c32 = lambda a: np.ascontiguousarray(a, dtype=np.float32)
    cbf = lambda a: np.ascontiguousarray(np.asarray(a, dtype=np.float32).astype(BF16))

    wq_full = {"v": f(Wqv), "a0": f(Wqa0), "a1": f(Wqa1)}
    wk_full = {"v": f(Wkv), "a0": f(Wka0), "a1": f(Wka1)}
    bq_full = {"v": f(bqv), "a0": f(bqa0), "a1": f(bqa1)}
    bk_full = {"v": f(bkv), "a0": f(bka0), "a1": f(bka1)}

    xts = {}
    maskABs = {}
    for b in range(B):
        xts[b] = {
            "v": cbf(e_v[b].T),
            "a0": cbf(e_a0[b].T),
            "a1": cbf(e_a1[b].T),
        }
        mT = f(attn_mask[b, 0]).T * (1.0 / SCALE)
        # diagonal-band mask patterns: A = (kt == 2*qb), B = (kt == 2*qb+1)
        maskABs[b] = cbf(np.stack([mT[0:128, 0:256], mT[128:256, 0:256]], axis=1))

    def fold_slice(w, S):
        # [D, C] slice -> [NQF, D, 128] fold-major
        ws = np.asarray(w[:, S], dtype=np.float32)  # [D, C]
        return np.ascontiguousarray(
            ws.reshape(D, NQF, 128).transpose(1, 0, 2).astype(BF16)
        )

    Wout_f = f(Wout).astype(np.float32)
    bout_f = f(bout).astype(np.float32)
    # [128 p, 8 cf, D] with cf = s*4+f mapping Wout rows s*512+f*128+p
    wout8 = np.ascontiguousarray(
        Wout_f.reshape(2, NQF, 128, D).transpose(2, 0, 1, 3).astype(BF16)
    ).reshape(128, 8, D)

    in_maps = []
    for c in range(NCORES):
        b, hh = c // 2, c % 2
        S = slice(hh * C, (hh + 1) * C)
        m = {}
        for s in STREAMS:
            m[f"xt_{s}"] = xts[b][s]
            m[f"wq_{s}"] = fold_slice(wq_full[s], S)
            m[f"wk_{s}"] = fold_slice(wk_full[s], S)
            m[f"bq_{s}"] = c32(bq_full[s][S])
            m[f"bk_{s}"] = c32(bk_full[s][S])
        m["wv"] = cbf(f(Wvv)[:, S])
        m["bv"] = cbf(f(bvv)[S]).reshape(1, C)
        m["wout8"] = wout8
        m["maskAB"] = maskABs[b]
        selv = np.zeros((128, 2, 512), dtype=np.float32)
        selv[:, hh, :] = 1.0
        m["sel"] = cbf(selv)
        m["ev_res"] = c32(e_v[b, hh * 512 : (hh + 1) * 512, :] + bout_f[None, :])
        m["gamma"] = c32(f(ln_gamma)).reshape(1, D)
        m["beta"] = c32(f(ln_beta)).reshape(1, D)
        in_maps.append(m)

    nc = _get_nc()
    trace = bool(os.environ.get("KERNEL_TRACE"))
    res = run_bass_kernel_spmd(
        nc, in_maps, core_ids=list(range(NCORES)), trace=trace
    )
    LAST_RESULT = res

    out_full = np.empty((B, L, D), dtype=np.float32)
    for c in range(NCORES):
        b, hh = c // 2, c % 2
        out_full[b, hh * 512 : (hh + 1) * 512, :] = res.results[c]["out"]
    return out_full


# revision 22
# speedup vs baseline: 1.9167x; 1.0384x over previous
"""DiffMHA (differential multi-head attention) block on 8 TRN2 NeuronCores.

Problem: B=4, L=1024, D=1024, H=16 heads (DH=64). Three input streams
(e_v, e_a0, e_a1); Q/K projections per stream, scores summed across
streams, causal-masked softmax, context from the v-stream values,
out-projection + residual + LayerNorm.

Sharding: (batch, head-half) -> 8 cores. Core c handles batch c//2 and
heads (c%2)*8 .. (c%2)*8+8. Each core computes its 8 heads' Q/K/V
projections (512 of 1024 channels), scores + softmax + context. Fold
context halves (128 channels x 512 rows) are exchanged between the two
cores of a batch via per-fold pairwise AllToAll DURING attention; each
core then runs the out-projection for its own 512 sequence rows with
the full 1024-channel contraction, then residual + LayerNorm locally.
No end-of-kernel collective.

Key optimizations over the v1 kernel:
- Causal skipping: score tiles with kt*128 > qb*256+255 are fully
  masked and skipped entirely (exp == 0 exactly); only diagonal-band
  tiles get the mask add. Attention matmul work drops ~40%.
- Stream packing: q/k of streams v and a0 are packed onto 128
  partitions (64 chans each) at projection-eviction time (partition-
  shifted PSUM->SBUF copies), so their two 64-deep score matmuls fuse
  into one 128-deep matmul; stream a1 stays a 64-deep accumulate.
- PSUM bank alternation: consecutive PE matmuls never accumulate into
  the same PSUM bank back-to-back (V-proj lf pairs, per-head score
  tiles, ctx of 2 heads, out-proj chains are interleaved), which keeps
  the PE at its ~216ns/512-col pipelined rate instead of ~430ns.
- Softmax normalization deferred past ctx accumulation via the extra
  ones-row of V (unchanged), reciprocal+broadcast per (head, q-half).
- DMA order: xt_v + wv first so the PE starts ~10us in, not ~46us.
"""

import os
import sys
import types

import ml_dtypes
import numpy as np

B, L, D, H = 4, 1024, 1024, 16
DH = D // H
HPC = H // 2  # heads per core
C = HPC * DH  # channels per core (512)
SCALE = float(1.0 / np.sqrt(DH))
EPS = 1e-12
NCORES = 8
BF16 = ml_dtypes.bfloat16


def _install_ntff_hook():
    """Recreate antenv.axon_hooks (absent in this image) so
    run_bass_kernel_spmd(trace=True) can capture NTFF profiles."""
    if "antenv.axon_hooks" in sys.modules:
        return
    try:
        from trn_agent_boot.trn_boot import _ntff_profile_via_ctypes

        hook = _ntff_profile_via_ctypes("/opt/axon/libaxon_pjrt.so")
    except Exception:
        hook = None
    mod = types.ModuleType("antenv.axon_hooks")
    mod.get_axon_ntff_profile_hook = lambda: hook
    mod.set_axon_ntff_profile_hook = lambda h: None
    sys.modules["antenv.axon_hooks"] = mod


_install_ntff_hook()

import concourse.bass as bass  # noqa: E402
import concourse.mybir as mybir  # noqa: E402
import concourse.tile as tile  # noqa: E402
from concourse import bacc  # noqa: E402
from concourse.bass_utils import run_bass_kernel_spmd  # noqa: E402

F32 = mybir.dt.float32
BF = mybir.dt.bfloat16
AF = mybir.ActivationFunctionType
ALU = mybir.AluOpType

_NC_CACHE = {}
LAST_RESULT = None

NQF = C // 128  # 4 channel folds per stream (2 heads each)
NLT = L // 128  # 8 l-tiles
NDT = D // 128  # 8 d-tiles (contraction)
NKT = L // 128  # 8 k-tiles
NRF = (L // 2) // 128  # 4 row tiles for out-proj/LN
STREAMS = ("v", "a0", "a1")
GROUPS = [[0, 1], [2, 3], [4, 5], [6, 7]]


def build_nc():
    nc = bacc.Bacc("TRN2", target_bir_lowering=False, debug=False, num_devices=NCORES)

    # ---- DRAM parameters (per-core shards, host-prepped) ----
    xt = {s: nc.declare_dram_parameter(f"xt_{s}", [D, L], BF, isOutput=False) for s in STREAMS}
    wq = {s: nc.declare_dram_parameter(f"wq_{s}", [NQF, D, 128], BF, isOutput=False) for s in STREAMS}
    wk = {s: nc.declare_dram_parameter(f"wk_{s}", [NQF, D, 128], BF, isOutput=False) for s in STREAMS}
    wv = nc.declare_dram_parameter("wv", [D, C], BF, isOutput=False)
    wout8 = nc.declare_dram_parameter("wout8", [128, 8, D], BF, isOutput=False)
    bq = {s: nc.declare_dram_parameter(f"bq_{s}", [C], F32, isOutput=False) for s in STREAMS}
    bk = {s: nc.declare_dram_parameter(f"bk_{s}", [C], F32, isOutput=False) for s in STREAMS}
    bv = nc.declare_dram_parameter("bv", [1, C], BF, isOutput=False)
    maskAB = nc.declare_dram_parameter("maskAB", [128, 2, 256], BF, isOutput=False)
    sel = nc.declare_dram_parameter("sel", [128, 2, 512], BF, isOutput=False)
    ev_res = nc.declare_dram_parameter("ev_res", [L // 2, D], F32, isOutput=False)
    gamma = nc.declare_dram_parameter("gamma", [1, D], F32, isOutput=False)
    beta = nc.declare_dram_parameter("beta", [1, D], F32, isOutput=False)
    out = nc.declare_dram_parameter("out", [L // 2, D], F32, isOutput=True)


    with tile.TileContext(nc) as tc:
        with (
            tc.tile_pool(name="persist", bufs=1) as persist,
            tc.tile_pool(name="xtp", bufs=1) as xtp,
            tc.tile_pool(name="wf", bufs=10) as wf,
            tc.tile_pool(name="qkf", bufs=2) as qkf,
            tc.tile_pool(name="small", bufs=4) as small,
            tc.tile_pool(name="attn", bufs=4) as attn_pool,
            tc.tile_pool(name="ln", bufs=3) as ln_pool,
            tc.tile_pool(name="evp", bufs=4) as evp,
            tc.tile_pool(name="ctxf", bufs=2) as ctxf_pool,
            tc.tile_pool(name="proj_ps", bufs=3, space="PSUM") as proj_ps,
            tc.tile_pool(name="sc_ps", bufs=3, space="PSUM") as sc_ps,
            tc.tile_pool(name="ctx_ps", bufs=2, space="PSUM") as ctx_ps,
            tc.tile_pool(name="dram", bufs=1, space="DRAM") as dram,
        ):
            # ---- persistent SBUF tensors ----
            vnat = persist.tile([128, NLT, HPC, DH + 1], BF, tag="vnat")
            ctx_all = persist.tile([128, 8, L // 2], BF, tag="ctxall")
            mask_sb = persist.tile([128, 2, 256], BF, tag="maskAB")
            ones_b = persist.tile([1, L], BF, tag="ones")
            gb_bc = persist.tile([128, 2, D], F32, tag="gbbc")
            bv_sb = persist.tile([1, C], BF, tag="bvsb")
            wv_sb = persist.tile([128, NDT, C], BF, tag="wvsb")
            wout_sb = persist.tile([128, 8, D], BF, tag="woutsb")
            eps_sb = persist.tile([128, 1], F32, tag="eps")
            bq_sb = {
                s: persist.tile([128, NQF], F32, tag=f"bq{s}", name=f"bq_sb_{s}")
                for s in STREAMS
            }
            bk_sb = {
                s: persist.tile([128, NQF], F32, tag=f"bk{s}", name=f"bk_sb_{s}")
                for s in STREAMS
            }

            # ---- fold weight loader (lazy, cached) ----
            wf_cache = {}

            def load_wf(s, ff):
                if (s, ff) in wf_cache:
                    return wf_cache[(s, ff)]
                wq_t = wf.tile([128, NDT, 128], BF, tag="w", name=f"wq_{s}{ff}")
                wk_t = wf.tile([128, NDT, 128], BF, tag="w", name=f"wk_{s}{ff}")
                nc.sync.dma_start(
                    out=wq_t[:, :, :],
                    in_=wq[s][ff, :, :].rearrange("(dt p) c -> p dt c", p=128),
                )
                nc.sync.dma_start(
                    out=wk_t[:, :, :],
                    in_=wk[s][ff, :, :].rearrange("(dt p) c -> p dt c", p=128),
                )
                wf_cache[(s, ff)] = (wq_t, wk_t)
                return wq_t, wk_t

            # ---- preamble DMAs, in critical-path order: V-proj deps
            #      first, then fold-0 Q/K weights interleaved with the
            #      remaining embeddings; big late-use tensors (wout, ev,
            #      gamma/beta) are deferred into the fold loop. ----
            xt_sb = {}
            t = xtp.tile([128, NDT, L], BF, tag="xtv", name="xt_sb_v")
            nc.sync.dma_start(
                out=t[:, :, :], in_=xt["v"][:, :].rearrange("(dt p) l -> p dt l", p=128)
            )
            xt_sb["v"] = t
            nc.sync.dma_start(
                out=wv_sb[:, :, :], in_=wv[:, :].rearrange("(dt p) c -> p dt c", p=128)
            )
            nc.sync.dma_start(out=bv_sb[:, :], in_=bv[:, :])
            load_wf("v", 0)
            for s in ("a0", "a1"):
                t = xtp.tile([128, NDT, L], BF, tag=f"xt{s}", name=f"xt_sb_{s}")
                nc.sync.dma_start(
                    out=t[:, :, :],
                    in_=xt[s][:, :].rearrange("(dt p) l -> p dt l", p=128),
                )
                xt_sb[s] = t
                load_wf(s, 0)

            nc.vector.memset(ones_b[:, :], 1.0)
            nc.vector.memset(eps_sb[:, :], EPS)
            nc.vector.memset(vnat[:, :, :, DH : DH + 1], 1.0)

            nc.sync.dma_start(out=mask_sb[:, :, :], in_=maskAB[:, :, :])
            sel_sb = persist.tile([128, 2, 512], BF, tag="sel")
            nc.sync.dma_start(out=sel_sb[:, :, :], in_=sel[:, :, :])
            for s in STREAMS:
                nc.sync.dma_start(
                    out=bq_sb[s][:, :], in_=bq[s][:].rearrange("(f p) -> p f", p=128)
                )
                nc.sync.dma_start(
                    out=bk_sb[s][:, :], in_=bk[s][:].rearrange("(f p) -> p f", p=128)
                )
            ev_sb = []

            # ---- V projection: natural [l, c] layout + ones column.
            #      lf pairs interleaved so consecutive matmuls alternate
            #      PSUM banks. ----
            for pair in range(NLT // 2):
                lf0, lf1 = 2 * pair, 2 * pair + 1
                psA = sc_ps.tile([128, C], F32, tag="sc")
                psB = sc_ps.tile([128, C], F32, tag="sc")
                for dt in range(NDT):
                    for lf, ps in ((lf0, psA), (lf1, psB)):
                        nc.tensor.matmul(
                            ps[:, :],
                            xt_sb["v"][:, dt, lf * 128 : (lf + 1) * 128],
                            wv_sb[:, dt, :],
                            start=(dt == 0),
                            stop=False,
                        )
                for lf, ps in ((lf0, psA), (lf1, psB)):
                    nc.tensor.matmul(
                        ps[:, :],
                        ones_b[:, lf * 128 : (lf + 1) * 128],
                        bv_sb[:, :],
                        start=False,
                        stop=True,
                    )
                nc.scalar.copy(vnat[:, lf0, :, 0:DH], psA[:, :])
                nc.scalar.copy(vnat[:, lf1, :, 0:DH], psB[:, :])

            # ---- fold-major main loop ----
            for f in range(NQF):
                # -- Q/K projections for all three streams --
                # packed tiles: partitions [0:64] = stream v chans of the
                # head, [64:128] = stream a0 chans; a1 keeps fold layout.
                qpk = [
                    qkf.tile([128, L], BF, tag=f"qpk{hh}", name=f"qpk{hh}_{f}")
                    for hh in range(2)
                ]
                kpk = [
                    qkf.tile([128, L], BF, tag=f"kpk{hh}", name=f"kpk{hh}_{f}")
                    for hh in range(2)
                ]
                qa1 = qkf.tile([128, L], BF, tag="qa1", name=f"qa1_{f}")
                ka1 = qkf.tile([128, L], BF, tag="ka1", name=f"ka1_{f}")

                # stage late-use loads into fold slots so they don't
                # compete with the critical-path preamble DMAs
                if f == 1:
                    nc.sync.dma_start(out=wout_sb[:, :, :], in_=wout8[:, :, :])
                if f == 2:
                    for rf in range(NRF):
                        t = evp.tile([128, D], F32, tag="ev", name=f"ev{rf}")
                        nc.sync.dma_start(
                            out=t[:, :], in_=ev_res[rf * 128 : (rf + 1) * 128, :]
                        )
                        ev_sb.append(t)
                if f == 3:
                    gsb = small.tile([1, D], F32, tag="gsb", bufs=1)
                    bsb = small.tile([1, D], F32, tag="bsb", bufs=1)
                    nc.sync.dma_start(out=gsb[:, :], in_=gamma[:, :])
                    nc.sync.dma_start(out=bsb[:, :], in_=beta[:, :])
                    nc.gpsimd.partition_broadcast(gb_bc[:, 0, :], gsb[:, :])
                    nc.gpsimd.partition_broadcast(gb_bc[:, 1, :], bsb[:, :])

                for s in STREAMS:
                    wq_t, wk_t = load_wf(s, f)
                    if f + 1 < NQF:
                        load_wf(s, f + 1)  # prefetch next fold's weights
                    for w_t, b_t, pk, a1t in (
                        (wq_t, bq_sb[s], qpk, qa1),
                        (wk_t, bk_sb[s], kpk, ka1),
                    ):
                        ps = [
                            proj_ps.tile([128, 512], F32, tag="proj", name=f"pp{lh}")
                            for lh in range(2)
                        ]
                        for dt in range(NDT):
                            for lh in range(2):
                                nc.tensor.matmul(
                                    ps[lh][:, :],
                                    w_t[:, dt, :],
                                    xt_sb[s][:, dt, lh * 512 : (lh + 1) * 512],
                                    start=(dt == 0),
                                    stop=(dt == NDT - 1),
                                )
                        for lh in range(2):
                            lsl = slice(lh * 512, (lh + 1) * 512)
                            if s == "a1":
                                nc.scalar.activation(
                                    a1t[:, lsl], ps[lh][:, :], AF.Identity,
                                    bias=b_t[:, f : f + 1],
                                )
                            else:
                                off = 0 if s == "v" else 64
                                for hh in range(2):
                                    nc.scalar.activation(
                                        pk[hh][off : off + 64, lsl],
                                        ps[lh][hh * 64 : hh * 64 + 64, :],
                                        AF.Identity,
                                        bias=b_t[hh * 64 : hh * 64 + 64, f : f + 1],
                                    )

                # -- attention for heads 2f (A: parts 0:64) and 2f+1 (B) --
                ctxf = ctxf_pool.tile([128, L], BF, tag="ctxf", name=f"ctxf{f}")
                for qh in range(2):
                    cps = [
                        ctx_ps.tile([DH + 1, 512], F32, tag="ctx", name=f"cps{i}")
                        for i in range(2)
                    ]
                    n_kt = 4 * qh + 4  # live k-tiles for this q-half
                    last_live = (4 * qh + 1, 4 * qh + 3)  # per qb half
                    sps_at = {}

                    def emit_scores(kt):
                        sps = [
                            sc_ps.tile([128, 512], F32, tag="sc", name=f"sps{i}")
                            for i in range(2)
                        ]
                        halves = [0, 1] if kt <= 4 * qh + 1 else [1]
                        for qbh in halves:
                            qsl = slice(qh * 512 + qbh * 256, qh * 512 + qbh * 256 + 256)
                            osl = slice(qbh * 256, qbh * 256 + 256)
                            ksl = slice(kt * 128, (kt + 1) * 128)
                            for hh in range(2):
                                nc.tensor.matmul(
                                    sps[hh][:, osl],
                                    kpk[hh][:, ksl],
                                    qpk[hh][:, qsl],
                                    start=True,
                                    stop=False,
                                )
                            for hh in range(2):
                                p0 = hh * 64
                                nc.tensor.matmul(
                                    sps[hh][:, osl],
                                    ka1[p0 : p0 + 64, ksl],
                                    qa1[p0 : p0 + 64, qsl],
                                    start=False,
                                    stop=True,
                                )
                        # mask only on diagonal-band halves
                        for qbh in halves:
                            qb = 2 * qh + qbh
                            if kt in (2 * qb, 2 * qb + 1):
                                osl = slice(qbh * 256, qbh * 256 + 256)
                                for hh in range(2):
                                    nc.vector.tensor_add(
                                        sps[hh][:, osl],
                                        sps[hh][:, osl],
                                        mask_sb[:, kt % 2, :],
                                    )
                        # exp -> bf16 attn tiles (dead qb0 half zeroed so
                        # the full-width ctx matmul accumulates one group
                        # per PSUM bank)
                        at = [
                            attn_pool.tile([128, 512], BF, tag="attn", name=f"at{i}")
                            for i in range(2)
                        ]
                        full = kt <= 4 * qh + 1
                        esl = slice(0, 512) if full else slice(256, 512)
                        for hh in range(2):
                            if not full:
                                nc.vector.memset(at[hh][:, 0:256], 0.0)
                            nc.scalar.activation(
                                at[hh][:, esl], sps[hh][:, esl], AF.Exp, scale=SCALE
                            )
                        sps_at[kt] = at

                    def emit_ctx(kt):
                        at = sps_at.pop(kt)
                        for hh in range(2):
                            h = 2 * f + hh
                            nc.tensor.matmul(
                                cps[hh][:, :],
                                vnat[:, kt, h, :],
                                at[hh][:, :],
                                start=(kt == 0),
                                stop=(kt == n_kt - 1),
                            )

                    prev = None
                    for kt in range(n_kt):
                        emit_scores(kt)
                        if prev is not None:
                            emit_ctx(prev)
                        prev = kt
                    emit_ctx(prev)

                    # normalize: divide ctx rows by the attn row-sums that
                    # accumulated in psum row DH
                    for hh in range(2):
                        p0 = hh * 64
                        inv = small.tile([1, 512], F32, tag="inv", bufs=2)
                        nc.vector.reciprocal(inv[:, :], cps[hh][DH : DH + 1, :])
                        inv_bc = small.tile([64, 512], F32, tag="invbc", bufs=2)
                        nc.gpsimd.partition_broadcast(inv_bc[:, :], inv[:, :])
                        nc.vector.tensor_mul(
                            ctxf[p0 : p0 + 64, qh * 512 : (qh + 1) * 512],
                            cps[hh][0:DH, :],
                            inv_bc[:, :],
                        )

                # -- exchange fold ctx halves with the pair core.
                # AllToAll isn't available for 2-core groups, so emulate it
                # with a ReduceScatter over [dest d][chan-slot s] staging
                # where slot s != own-half is zeroed via the host-provided
                # 0/1 `sel` tensor (x + 0 is exact in bf16). Rank d then
                # receives [ctx_half0, ctx_half1] for its own rows. --
                cxs = ctxf_pool.tile([128, 2, 2, 512], BF, tag="cxs", name=f"cxs{f}", bufs=1)
                for d2 in range(2):
                    for s2 in range(2):
                        nc.vector.tensor_mul(
                            cxs[:, d2, s2, :],
                            ctxf[:, d2 * 512 : (d2 + 1) * 512],
                            sel_sb[:, s2, :],
                        )
                cx_in = dram.tile(
                    [2, 2, 128, 512], BF, tag=f"cxin{f}", name=f"cxin{f}"
                )
                cx_out = dram.tile(
                    [2, 128, 512], BF, tag=f"cxout{f}", name=f"cxout{f}"
                )
                for d2 in range(2):
                    for s2 in range(2):
                        nc.sync.dma_start(
                            out=cx_in[d2, s2, :, :], in_=cxs[:, d2, s2, :]
                        )
                nc.gpsimd.collective_compute(
                    "ReduceScatter",
                    ALU.add,
                    replica_groups=GROUPS,
                    ins=[cx_in.opt()],
                    outs=[cx_out.opt()],
                )
                for s2 in range(2):
                    nc.sync.dma_start(
                        out=ctx_all[:, s2 * NQF + f, :], in_=cx_out[s2, :, :]
                    )

            # ---- out-projection over full 1024 channels for own rows ----
            # 8 chains (lt, dh); consecutive chains interleaved per cf so
            # PSUM banks alternate.
            x_sb = [None] * NRF
            for cp in range(NRF):  # chain pair = one lt row tile
                lt = cp
                lsl = slice(lt * 128, (lt + 1) * 128)
                ops = [
                    sc_ps.tile([128, 512], F32, tag="sc", name=f"ops{i}")
                    for i in range(2)
                ]
                for cf in range(8):
                    for dh in range(2):
                        nc.tensor.matmul(
                            ops[dh][:, :],
                            ctx_all[:, cf, lsl],
                            wout_sb[:, cf, dh * 512 : (dh + 1) * 512],
                            start=(cf == 0),
                            stop=(cf == 7),
                        )
                xt_ = ln_pool.tile([128, D], F32, tag="x", name=f"x{lt}")
                nc.scalar.copy(xt_[:, 0:512], ops[0][:, :])
                nc.scalar.copy(xt_[:, 512:1024], ops[1][:, :])
                x_sb[lt] = xt_

                # ---- residual + LayerNorm on this row tile ----
                nc.vector.tensor_add(xt_[:, :], xt_[:, :], ev_sb[lt][:, :])
                stats = small.tile([128, 2, 6], F32, tag="stats")
                nc.vector.bn_stats(out=stats[:, 0, :], in_=xt_[:, 0:512])
                nc.vector.bn_stats(out=stats[:, 1, :], in_=xt_[:, 512:1024])
                mv = small.tile([128, 2], F32, tag="mv")
                nc.vector.bn_aggr(out=mv[:, :], in_=stats[:, :, :])
                std = small.tile([128, 1], F32, tag="std")
                nc.scalar.activation(std[:, :], mv[:, 1:2], AF.Sqrt, bias=eps_sb[:, :])
                rstd = small.tile([128, 1], F32, tag="rstd")
                nc.vector.reciprocal(rstd[:, :], std[:, :])
                negmb = small.tile([128, 1], F32, tag="negmb")
                nc.vector.scalar_tensor_tensor(
                    negmb[:, :],
                    mv[:, 0:1],
                    -1.0,
                    rstd[:, :],
                    op0=ALU.mult,
                    op1=ALU.mult,
                )
                nc.scalar.activation(
                    xt_[:, :],
                    xt_[:, :],
                    AF.Identity,
                    bias=negmb[:, :],
                    scale=rstd[:, :],
                )
                nc.vector.tensor_mul(xt_[:, :], xt_[:, :], gb_bc[:, 0, :])
                nc.vector.tensor_add(xt_[:, :], xt_[:, :], gb_bc[:, 1, :])
                nc.sync.dma_start(out=out[lsl, :], in_=xt_[:, :])

    nc.compile()
    return nc


def _get_nc():
    if "nc" not in _NC_CACHE:
        _NC_CACHE["nc"] = build_nc()
    return _NC_CACHE["nc"]


def kernel(
    e_v, e_a0, e_a1, Wqv, bqv, Wkv, bkv, Wvv, bvv,
    Wqa0, bqa0, Wka0, bka0, Wqa1, bqa1, Wka1, bka1,
    Wout, bout, ln_gamma, ln_beta, attn_mask,
):
    global LAST_RESULT
    f = np.asarray
    e_v, e_a0, e_a1 = f(e_v), f(e_a0), f(e_a1)
    attn_mask = f(attn_mask)
    c32 = lambda a: np.ascontiguousarray(a, dtype=np.float32)
    cbf = lambda a: np.ascontiguousarray(np.asarray(a, dtype=np.float32).astype(BF16))

    wq_full = {"v": f(Wqv), "a0": f(Wqa0), "a1": f(Wqa1)}
    wk_full = {"v": f(Wkv), "a0": f(Wka0), "a1": f(Wka1)}
    bq_full = {"v": f(bqv), "a0": f(bqa0), "a1": f(bqa1)}
    bk_full = {"v": f(bkv), "a0": f(bka0), "a1": f(bka1)}

    xts = {}
    maskABs = {}
    for b in range(B):
        xts[b] = {
            "v": cbf(e_v[b].T),
            "a0": cbf(e_a0[b].T),
            "a1": cbf(e_a1[b].T),
        }
        mT = f(attn_mask[b, 0]).T * (1.0 / SCALE)
        # diagonal-band mask patterns: A = (kt == 2*qb), B = (kt == 2*qb+1)
        maskABs[b] = cbf(np.stack([mT[0:128, 0:256], mT[128:256, 0:256]], axis=1))

    def fold_slice(w, S):
        # [D, C] slice -> [NQF, D, 128] fold-major
        ws = np.asarray(w[:, S], dtype=np.float32)  # [D, C]
        return np.ascontiguousarray(
            ws.reshape(D, NQF, 128).transpose(1, 0, 2).astype(BF16)
        )

    Wout_f = f(Wout).astype(np.float32)
    bout_f = f(bout).astype(np.float32)
    # [128 p, 8 cf, D] with cf = s*4+f mapping Wout rows s*512+f*128+p
    wout8 = np.ascontiguousarray(
        Wout_f.reshape(2, NQF, 128, D).transpose(2, 0, 1, 3).astype(BF16)
    ).reshape(128, 8, D)

    in_maps = []
    for c in range(NCORES):
        b, hh = c // 2, c % 2
        S = slice(hh * C, (hh + 1) * C)
        m = {}
        for s in STREAMS:
            m[f"xt_{s}"] = xts[b][s]
            m[f"wq_{s}"] = fold_slice(wq_full[s], S)
            m[f"wk_{s}"] = fold_slice(wk_full[s], S)
            m[f"bq_{s}"] = c32(bq_full[s][S])
            m[f"bk_{s}"] = c32(bk_full[s][S])
        m["wv"] = cbf(f(Wvv)[:, S])
        m["bv"] = cbf(f(bvv)[S]).reshape(1, C)
        m["wout8"] = wout8
        m["maskAB"] = maskABs[b]
        selv = np.zeros((128, 2, 512), dtype=np.float32)
        selv[:, hh, :] = 1.0
        m["sel"] = cbf(selv)
        m["ev_res"] = c32(e_v[b, hh * 512 : (hh + 1) * 512, :] + bout_f[None, :])
        m["gamma"] = c32(f(ln_gamma)).reshape(1, D)
        m["beta"] = c32(f(ln_beta)).reshape(1, D)
        in_maps.append(m)

    nc = _get_nc()
    trace = bool(os.environ.get("KERNEL_TRACE"))
    res = run_bass_kernel_spmd(
        nc, in_maps, core_ids=list(range(NCORES)), trace=trace
    )
    LAST_RESULT = res

    out_full = np.empty((B, L, D), dtype=np.float32)
    for c in range(NCORES):
        b, hh = c // 2, c % 2
        out_full[b, hh * 512 : (hh + 1) * 512, :] = res.results[c]["out"]
    return out_full


# revision 25
# speedup vs baseline: 1.9772x; 1.0316x over previous
"""DiffMHA (differential multi-head attention) block on 8 TRN2 NeuronCores.

Problem: B=4, L=1024, D=1024, H=16 heads (DH=64). Three input streams
(e_v, e_a0, e_a1); Q/K projections per stream, scores summed across
streams, causal-masked softmax, context from the v-stream values,
out-projection + residual + LayerNorm.

Sharding: (batch, head-half) -> 8 cores. Core c handles batch c//2 and
heads (c%2)*8 .. (c%2)*8+8. Each core computes its 8 heads' Q/K/V
projections (512 of 1024 channels), scores + softmax + context. Fold
context halves (128 channels x 512 rows) are exchanged between the two
cores of a batch via per-fold pairwise AllToAll DURING attention; each
core then runs the out-projection for its own 512 sequence rows with
the full 1024-channel contraction, then residual + LayerNorm locally.
No end-of-kernel collective.

Key optimizations over the v1 kernel:
- Causal skipping: score tiles with kt*128 > qb*256+255 are fully
  masked and skipped entirely (exp == 0 exactly); only diagonal-band
  tiles get the mask add. Attention matmul work drops ~40%.
- Stream packing: q/k of streams v and a0 are packed onto 128
  partitions (64 chans each) at projection-eviction time (partition-
  shifted PSUM->SBUF copies), so their two 64-deep score matmuls fuse
  into one 128-deep matmul; stream a1 stays a 64-deep accumulate.
- PSUM bank alternation: consecutive PE matmuls never accumulate into
  the same PSUM bank back-to-back (V-proj lf pairs, per-head score
  tiles, ctx of 2 heads, out-proj chains are interleaved), which keeps
  the PE at its ~216ns/512-col pipelined rate instead of ~430ns.
- Softmax normalization deferred past ctx accumulation via the extra
  ones-row of V (unchanged), reciprocal+broadcast per (head, q-half).
- DMA order: xt_v + wv first so the PE starts ~10us in, not ~46us.
"""

import os
import sys
import types

import ml_dtypes
import numpy as np

B, L, D, H = 4, 1024, 1024, 16
DH = D // H
HPC = H // 2  # heads per core
C = HPC * DH  # channels per core (512)
SCALE = float(1.0 / np.sqrt(DH))
EPS = 1e-12
NCORES = 8
BF16 = ml_dtypes.bfloat16


def _install_ntff_hook():
    """Recreate antenv.axon_hooks (absent in this image) so
    run_bass_kernel_spmd(trace=True) can capture NTFF profiles."""
    if "antenv.axon_hooks" in sys.modules:
        return
    try:
        from trn_agent_boot.trn_boot import _ntff_profile_via_ctypes

        hook = _ntff_profile_via_ctypes("/opt/axon/libaxon_pjrt.so")
    except Exception:
        hook = None
    mod = types.ModuleType("antenv.axon_hooks")
    mod.get_axon_ntff_profile_hook = lambda: hook
    mod.set_axon_ntff_profile_hook = lambda h: None
    sys.modules["antenv.axon_hooks"] = mod


_install_ntff_hook()

import concourse.bass as bass  # noqa: E402
import concourse.mybir as mybir  # noqa: E402
import concourse.tile as tile  # noqa: E402
from concourse import bacc  # noqa: E402
from concourse.bass_utils import run_bass_kernel_spmd  # noqa: E402

F32 = mybir.dt.float32
BF = mybir.dt.bfloat16
AF = mybir.ActivationFunctionType
ALU = mybir.AluOpType

_NC_CACHE = {}
LAST_RESULT = None

NQF = C // 128  # 4 channel folds per stream (2 heads each)
NLT = L // 128  # 8 l-tiles
NDT = D // 128  # 8 d-tiles (contraction)
NKT = L // 128  # 8 k-tiles
NRF = (L // 2) // 128  # 4 row tiles for out-proj/LN
STREAMS = ("v", "a0", "a1")
GROUPS = [[0, 1], [2, 3], [4, 5], [6, 7]]


def build_nc():
    nc = bacc.Bacc("TRN2", target_bir_lowering=False, debug=False, num_devices=NCORES)

    # ---- DRAM parameters (per-core shards, host-prepped) ----
    xt = {s: nc.declare_dram_parameter(f"xt_{s}", [D, L], BF, isOutput=False) for s in STREAMS}
    wq = {s: nc.declare_dram_parameter(f"wq_{s}", [NQF, D, 128], BF, isOutput=False) for s in STREAMS}
    wk = {s: nc.declare_dram_parameter(f"wk_{s}", [NQF, D, 128], BF, isOutput=False) for s in STREAMS}
    wv = nc.declare_dram_parameter("wv", [D, C], BF, isOutput=False)
    wout8 = nc.declare_dram_parameter("wout8", [128, 8, D], BF, isOutput=False)
    bq = {s: nc.declare_dram_parameter(f"bq_{s}", [C], F32, isOutput=False) for s in STREAMS}
    bk = {s: nc.declare_dram_parameter(f"bk_{s}", [C], F32, isOutput=False) for s in STREAMS}
    bv = nc.declare_dram_parameter("bv", [1, C], BF, isOutput=False)
    maskAB = nc.declare_dram_parameter("maskAB", [128, 2, 256], BF, isOutput=False)
    sel = nc.declare_dram_parameter("sel", [128, 2, 512], BF, isOutput=False)
    ev_res = nc.declare_dram_parameter("ev_res", [L // 2, D], BF, isOutput=False)
    eye = nc.declare_dram_parameter("eye", [128, 128], BF, isOutput=False)
    gamma = nc.declare_dram_parameter("gamma", [1, D], F32, isOutput=False)
    beta = nc.declare_dram_parameter("beta", [1, D], F32, isOutput=False)
    out = nc.declare_dram_parameter("out", [L // 2, D], F32, isOutput=True)


    with tile.TileContext(nc) as tc:
        with (
            tc.tile_pool(name="persist", bufs=1) as persist,
            tc.tile_pool(name="xtp", bufs=1) as xtp,
            tc.tile_pool(name="wf", bufs=10) as wf,
            tc.tile_pool(name="qkf", bufs=2) as qkf,
            tc.tile_pool(name="small", bufs=4) as small,
            tc.tile_pool(name="attn", bufs=4) as attn_pool,
            tc.tile_pool(name="ln", bufs=4) as ln_pool,
            tc.tile_pool(name="evp", bufs=4) as evp,
            tc.tile_pool(name="ctxf", bufs=2) as ctxf_pool,
            tc.tile_pool(name="proj_ps", bufs=3, space="PSUM") as proj_ps,
            tc.tile_pool(name="sc_ps", bufs=3, space="PSUM") as sc_ps,
            tc.tile_pool(name="ctx_ps", bufs=2, space="PSUM") as ctx_ps,
            tc.tile_pool(name="dram", bufs=1, space="DRAM") as dram,
        ):
            # ---- persistent SBUF tensors ----
            vnat = persist.tile([128, NLT, HPC, DH + 1], BF, tag="vnat")
            ctx_all = persist.tile([128, 8, L // 2], BF, tag="ctxall")
            mask_sb = persist.tile([128, 2, 256], BF, tag="maskAB")
            ones_b = persist.tile([1, L], BF, tag="ones")
            gb_bc = persist.tile([128, 2, D], F32, tag="gbbc")
            bv_sb = persist.tile([1, C], BF, tag="bvsb")
            wv_sb = persist.tile([128, NDT, C], BF, tag="wvsb")
            wout_sb = persist.tile([128, 8, D], BF, tag="woutsb")
            eps_sb = persist.tile([128, 1], F32, tag="eps")
            eye_sb = persist.tile([128, 128], BF, tag="eye")
            bq_sb = {
                s: persist.tile([128, NQF], F32, tag=f"bq{s}", name=f"bq_sb_{s}")
                for s in STREAMS
            }
            bk_sb = {
                s: persist.tile([128, NQF], F32, tag=f"bk{s}", name=f"bk_sb_{s}")
                for s in STREAMS
            }

            # ---- fold weight loader (lazy, cached) ----
            wf_cache = {}

            def load_wf(s, ff):
                if (s, ff) in wf_cache:
                    return wf_cache[(s, ff)]
                wq_t = wf.tile([128, NDT, 128], BF, tag="w", name=f"wq_{s}{ff}")
                wk_t = wf.tile([128, NDT, 128], BF, tag="w", name=f"wk_{s}{ff}")
                nc.sync.dma_start(
                    out=wq_t[:, :, :],
                    in_=wq[s][ff, :, :].rearrange("(dt p) c -> p dt c", p=128),
                )
                nc.sync.dma_start(
                    out=wk_t[:, :, :],
                    in_=wk[s][ff, :, :].rearrange("(dt p) c -> p dt c", p=128),
                )
                wf_cache[(s, ff)] = (wq_t, wk_t)
                return wq_t, wk_t

            # ---- preamble DMAs, in critical-path order: V-proj deps
            #      first, then fold-0 Q/K weights interleaved with the
            #      remaining embeddings; big late-use tensors (wout, ev,
            #      gamma/beta) are deferred into the fold loop. ----
            xt_sb = {}
            t = xtp.tile([128, NDT, L], BF, tag="xtv", name="xt_sb_v")
            nc.sync.dma_start(
                out=t[:, :, :], in_=xt["v"][:, :].rearrange("(dt p) l -> p dt l", p=128)
            )
            xt_sb["v"] = t
            nc.sync.dma_start(
                out=wv_sb[:, :, :], in_=wv[:, :].rearrange("(dt p) c -> p dt c", p=128)
            )
            nc.sync.dma_start(out=bv_sb[:, :], in_=bv[:, :])
            load_wf("v", 0)
            for s in ("a0", "a1"):
                t = xtp.tile([128, NDT, L], BF, tag=f"xt{s}", name=f"xt_sb_{s}")
                nc.sync.dma_start(
                    out=t[:, :, :],
                    in_=xt[s][:, :].rearrange("(dt p) l -> p dt l", p=128),
                )
                xt_sb[s] = t
                load_wf(s, 0)

            nc.vector.memset(ones_b[:, :], 1.0)
            nc.vector.memset(eps_sb[:, :], EPS)
            nc.vector.memset(vnat[:, :, :, DH : DH + 1], 1.0)

            nc.sync.dma_start(out=mask_sb[:, :, :], in_=maskAB[:, :, :])
            sel_sb = persist.tile([128, 2, 512], BF, tag="sel")
            nc.sync.dma_start(out=sel_sb[:, :, :], in_=sel[:, :, :])
            nc.sync.dma_start(out=eye_sb[:, :], in_=eye[:, :])
            for s in STREAMS:
                nc.sync.dma_start(
                    out=bq_sb[s][:, :], in_=bq[s][:].rearrange("(f p) -> p f", p=128)
                )
                nc.sync.dma_start(
                    out=bk_sb[s][:, :], in_=bk[s][:].rearrange("(f p) -> p f", p=128)
                )
            ev_sb = []

            # ---- V projection: natural [l, c] layout + ones column.
            #      lf pairs interleaved so consecutive matmuls alternate
            #      PSUM banks. ----
            for pair in range(NLT // 2):
                lf0, lf1 = 2 * pair, 2 * pair + 1
                psA = sc_ps.tile([128, C], F32, tag="sc")
                psB = sc_ps.tile([128, C], F32, tag="sc")
                for dt in range(NDT):
                    for lf, ps in ((lf0, psA), (lf1, psB)):
                        nc.tensor.matmul(
                            ps[:, :],
                            xt_sb["v"][:, dt, lf * 128 : (lf + 1) * 128],
                            wv_sb[:, dt, :],
                            start=(dt == 0),
                            stop=False,
                        )
                for lf, ps in ((lf0, psA), (lf1, psB)):
                    nc.tensor.matmul(
                        ps[:, :],
                        ones_b[:, lf * 128 : (lf + 1) * 128],
                        bv_sb[:, :],
                        start=False,
                        stop=True,
                    )
                nc.scalar.copy(vnat[:, lf0, :, 0:DH], psA[:, :])
                nc.scalar.copy(vnat[:, lf1, :, 0:DH], psB[:, :])

            # ---- fold-major main loop ----
            for f in range(NQF):
                # -- Q/K projections for all three streams --
                # packed tiles: partitions [0:64] = stream v chans of the
                # head, [64:128] = stream a0 chans; a1 keeps fold layout.
                qpk = [
                    qkf.tile([128, L], BF, tag=f"qpk{hh}", name=f"qpk{hh}_{f}")
                    for hh in range(2)
                ]
                kpk = [
                    qkf.tile([128, L], BF, tag=f"kpk{hh}", name=f"kpk{hh}_{f}")
                    for hh in range(2)
                ]
                qa1 = qkf.tile([128, L], BF, tag="qa1", name=f"qa1_{f}")
                ka1 = qkf.tile([128, L], BF, tag="ka1", name=f"ka1_{f}")

                # stage late-use loads into fold slots so they don't
                # compete with the critical-path preamble DMAs
                if f == 1:
                    nc.sync.dma_start(out=wout_sb[:, :, :], in_=wout8[:, :, :])
                if f == 2:
                    for rf in range(NRF):
                        t = evp.tile([128, D], BF, tag="ev", name=f"ev{rf}")
                        nc.sync.dma_start(
                            out=t[:, :], in_=ev_res[rf * 128 : (rf + 1) * 128, :]
                        )
                        ev_sb.append(t)
                if f == 3:
                    gsb = small.tile([1, D], F32, tag="gsb", bufs=1)
                    bsb = small.tile([1, D], F32, tag="bsb", bufs=1)
                    nc.sync.dma_start(out=gsb[:, :], in_=gamma[:, :])
                    nc.sync.dma_start(out=bsb[:, :], in_=beta[:, :])
                    nc.gpsimd.partition_broadcast(gb_bc[:, 0, :], gsb[:, :])
                    nc.gpsimd.partition_broadcast(gb_bc[:, 1, :], bsb[:, :])

                for s in STREAMS:
                    wq_t, wk_t = load_wf(s, f)
                    if f + 1 < NQF:
                        load_wf(s, f + 1)  # prefetch next fold's weights
                    for w_t, b_t, pk, a1t in (
                        (wq_t, bq_sb[s], qpk, qa1),
                        (wk_t, bk_sb[s], kpk, ka1),
                    ):
                        ps = [
                            proj_ps.tile([128, 512], F32, tag="proj", name=f"pp{lh}")
                            for lh in range(2)
                        ]
                        for dt in range(NDT):
                            for lh in range(2):
                                nc.tensor.matmul(
                                    ps[lh][:, :],
                                    w_t[:, dt, :],
                                    xt_sb[s][:, dt, lh * 512 : (lh + 1) * 512],
                                    start=(dt == 0),
                                    stop=(dt == NDT - 1),
                                )
                        for lh in range(2):
                            lsl = slice(lh * 512, (lh + 1) * 512)
                            if s == "a1":
                                nc.scalar.activation(
                                    a1t[:, lsl], ps[lh][:, :], AF.Identity,
                                    bias=b_t[:, f : f + 1],
                                )
                            else:
                                off = 0 if s == "v" else 64
                                for hh in range(2):
                                    nc.scalar.activation(
                                        pk[hh][off : off + 64, lsl],
                                        ps[lh][hh * 64 : hh * 64 + 64, :],
                                        AF.Identity,
                                        bias=b_t[hh * 64 : hh * 64 + 64, f : f + 1],
                                    )

                # -- attention for heads 2f (A: parts 0:64) and 2f+1 (B) --
                ctxf = ctxf_pool.tile([128, L], BF, tag="ctxf", name=f"ctxf{f}")
                cxs = ctxf_pool.tile(
                    [128, 2, 2, 512], BF, tag="cxs", name=f"cxs{f}", bufs=1
                )
                cx_in = dram.tile(
                    [2, 2, 128, 512], BF, tag=f"cxin{f}", name=f"cxin{f}"
                )
                for qh in range(2):
                    cps = [
                        ctx_ps.tile([DH + 1, 512], F32, tag="ctx", name=f"cps{i}")
                        for i in range(2)
                    ]
                    n_kt = 4 * qh + 4  # live k-tiles for this q-half
                    last_live = (4 * qh + 1, 4 * qh + 3)  # per qb half
                    sps_at = {}

                    def emit_scores(kt):
                        sps = [
                            sc_ps.tile([128, 512], F32, tag="sc", name=f"sps{i}")
                            for i in range(2)
                        ]
                        # PE issue is ~216ns/instr regardless of width, so
                        # use one full 512-col matmul pair when both q
                        # halves are live; 256-col only on the causal edge.
                        full = kt <= 4 * qh + 1
                        qsl = (
                            slice(qh * 512, qh * 512 + 512)
                            if full
                            else slice(qh * 512 + 256, qh * 512 + 512)
                        )
                        osl = slice(0, 512) if full else slice(256, 512)
                        ksl = slice(kt * 128, (kt + 1) * 128)
                        for hh in range(2):
                            nc.tensor.matmul(
                                sps[hh][:, osl],
                                kpk[hh][:, ksl],
                                qpk[hh][:, qsl],
                                start=True,
                                stop=False,
                            )
                        for hh in range(2):
                            p0 = hh * 64
                            nc.tensor.matmul(
                                sps[hh][:, osl],
                                ka1[p0 : p0 + 64, ksl],
                                qa1[p0 : p0 + 64, qsl],
                                start=False,
                                stop=True,
                            )
                        # mask only on diagonal-band halves
                        for qbh in range(2):
                            qb = 2 * qh + qbh
                            if kt in (2 * qb, 2 * qb + 1):
                                msl = slice(qbh * 256, qbh * 256 + 256)
                                for hh in range(2):
                                    nc.vector.tensor_add(
                                        sps[hh][:, msl],
                                        sps[hh][:, msl],
                                        mask_sb[:, kt % 2, :],
                                    )
                        # exp -> bf16 attn tiles (dead qb0 half zeroed so
                        # the full-width ctx matmul accumulates one group
                        # per PSUM bank)
                        at = [
                            attn_pool.tile([128, 512], BF, tag="attn", name=f"at{i}")
                            for i in range(2)
                        ]
                        esl = osl
                        for hh in range(2):
                            if not full:
                                nc.vector.memset(at[hh][:, 0:256], 0.0)
                            nc.scalar.activation(
                                at[hh][:, esl], sps[hh][:, esl], AF.Exp, scale=SCALE
                            )
                        sps_at[kt] = at

                    def emit_ctx(kt):
                        at = sps_at.pop(kt)
                        for hh in range(2):
                            h = 2 * f + hh
                            nc.tensor.matmul(
                                cps[hh][:, :],
                                vnat[:, kt, h, :],
                                at[hh][:, :],
                                start=(kt == 0),
                                stop=(kt == n_kt - 1),
                            )

                    prev = None
                    for kt in range(n_kt):
                        emit_scores(kt)
                        if prev is not None:
                            emit_ctx(prev)
                        prev = kt
                    emit_ctx(prev)

                    # normalize: divide ctx rows by the attn row-sums that
                    # accumulated in psum row DH
                    for hh in range(2):
                        p0 = hh * 64
                        inv = small.tile([1, 512], F32, tag="inv", bufs=2)
                        nc.vector.reciprocal(inv[:, :], cps[hh][DH : DH + 1, :])
                        inv_bc = small.tile([64, 512], F32, tag="invbc", bufs=2)
                        nc.gpsimd.partition_broadcast(inv_bc[:, :], inv[:, :])
                        nc.vector.tensor_mul(
                            ctxf[p0 : p0 + 64, qh * 512 : (qh + 1) * 512],
                            cps[hh][0:DH, :],
                            inv_bc[:, :],
                        )

                    # stage this q-half (= dest-rank chunk) for the
                    # exchange as soon as it is normalized
                    for s2 in range(2):
                        nc.vector.tensor_mul(
                            cxs[:, qh, s2, :],
                            ctxf[:, qh * 512 : (qh + 1) * 512],
                            sel_sb[:, s2, :],
                        )
                        nc.sync.dma_start(
                            out=cx_in[qh, s2, :, :], in_=cxs[:, qh, s2, :]
                        )

                # -- exchange fold ctx halves with the pair core.
                # AllToAll isn't available for 2-core groups, so emulate it
                # with a ReduceScatter over [dest d][chan-slot s] staging
                # where slot s != own-half is zeroed via the host-provided
                # 0/1 `sel` tensor (x + 0 is exact in bf16). Rank d then
                # receives [ctx_half0, ctx_half1] for its own rows. --
                cx_out = dram.tile(
                    [2, 128, 512], BF, tag=f"cxout{f}", name=f"cxout{f}"
                )
                nc.gpsimd.collective_compute(
                    "ReduceScatter",
                    ALU.add,
                    replica_groups=GROUPS,
                    ins=[cx_in.opt()],
                    outs=[cx_out.opt()],
                )
                for s2 in range(2):
                    nc.sync.dma_start(
                        out=ctx_all[:, s2 * NQF + f, :], in_=cx_out[s2, :, :]
                    )

            # ---- out-projection over full 1024 channels for own rows ----
            # 8 chains (lt, dh); consecutive chains interleaved per cf so
            # PSUM banks alternate.
            x_sb = [None] * NRF
            for cp in range(NRF):  # chain pair = one lt row tile
                lt = cp
                lsl = slice(lt * 128, (lt + 1) * 128)
                ops = [
                    sc_ps.tile([128, 512], F32, tag="sc", name=f"ops{i}")
                    for i in range(2)
                ]
                for cf in range(8):
                    for dh in range(2):
                        nc.tensor.matmul(
                            ops[dh][:, :],
                            ctx_all[:, cf, lsl],
                            wout_sb[:, cf, dh * 512 : (dh + 1) * 512],
                            start=(cf == 0),
                            stop=False,
                        )
                # residual (e_v + bout, bf16) added on the PE via an
                # identity-stationary matmul
                for dh in range(2):
                    nc.tensor.matmul(
                        ops[dh][:, :],
                        eye_sb[:, :],
                        ev_sb[lt][:, dh * 512 : (dh + 1) * 512],
                        start=False,
                        stop=True,
                    )
                xt_ = ln_pool.tile([128, D], F32, tag="x", name=f"x{lt}")
                nc.scalar.copy(xt_[:, 0:512], ops[0][:, :])
                nc.scalar.copy(xt_[:, 512:1024], ops[1][:, :])
                x_sb[lt] = xt_

                # ---- LayerNorm on this row tile (residual already in) ----
                stats = small.tile([128, 2, 6], F32, tag="stats")
                nc.vector.bn_stats(out=stats[:, 0, :], in_=xt_[:, 0:512])
                nc.vector.bn_stats(out=stats[:, 1, :], in_=xt_[:, 512:1024])
                mv = small.tile([128, 2], F32, tag="mv")
                nc.vector.bn_aggr(out=mv[:, :], in_=stats[:, :, :])
                std = small.tile([128, 1], F32, tag="std")
                nc.scalar.activation(std[:, :], mv[:, 1:2], AF.Sqrt, bias=eps_sb[:, :])
                rstd = small.tile([128, 1], F32, tag="rstd")
                nc.vector.reciprocal(rstd[:, :], std[:, :])
                negmb = small.tile([128, 1], F32, tag="negmb")
                nc.vector.scalar_tensor_tensor(
                    negmb[:, :],
                    mv[:, 0:1],
                    -1.0,
                    rstd[:, :],
                    op0=ALU.mult,
                    op1=ALU.mult,
                )
                nc.scalar.activation(
                    xt_[:, :],
                    xt_[:, :],
                    AF.Identity,
                    bias=negmb[:, :],
                    scale=rstd[:, :],
                )
                nc.vector.tensor_mul(xt_[:, :], xt_[:, :], gb_bc[:, 0, :])
                nc.vector.tensor_add(xt_[:, :], xt_[:, :], gb_bc[:, 1, :])
                nc.sync.dma_start(out=out[lsl, :], in_=xt_[:, :])

    nc.compile()
    return nc


def _get_nc():
    if "nc" not in _NC_CACHE:
        _NC_CACHE["nc"] = build_nc()
    return _NC_CACHE["nc"]


def kernel(
    e_v, e_a0, e_a1, Wqv, bqv, Wkv, bkv, Wvv, bvv,
    Wqa0, bqa0, Wka0, bka0, Wqa1, bqa1, Wka1, bka1,
    Wout, bout, ln_gamma, ln_beta, attn_mask,
):
    global LAST_RESULT
    f = np.asarray
    e_v, e_a0, e_a1 = f(e_v), f(e_a0), f(e_a1)
    attn_mask = f(attn_mask)
    c32 = lambda a: np.ascontiguousarray(a, dtype=np.float32)
    cbf = lambda a: np.ascontiguousarray(np.asarray(a, dtype=np.float32).astype(BF16))

    wq_full = {"v": f(Wqv), "a0": f(Wqa0), "a1": f(Wqa1)}
    wk_full = {"v": f(Wkv), "a0": f(Wka0), "a1": f(Wka1)}
    bq_full = {"v": f(bqv), "a0": f(bqa0), "a1": f(bqa1)}
    bk_full = {"v": f(bkv), "a0": f(bka0), "a1": f(bka1)}

    xts = {}
    maskABs = {}
    for b in range(B):
        xts[b] = {
            "v": cbf(e_v[b].T),
            "a0": cbf(e_a0[b].T),
            "a1": cbf(e_a1[b].T),
        }
        mT = f(attn_mask[b, 0]).T * (1.0 / SCALE)
        # diagonal-band mask patterns: A = (kt == 2*qb), B = (kt == 2*qb+1)
        maskABs[b] = cbf(np.stack([mT[0:128, 0:256], mT[128:256, 0:256]], axis=1))

    def fold_slice(w, S):
        # [D, C] slice -> [NQF, D, 128] fold-major
        ws = np.asarray(w[:, S], dtype=np.float32)  # [D, C]
        return np.ascontiguousarray(
            ws.reshape(D, NQF, 128).transpose(1, 0, 2).astype(BF16)
        )

    Wout_f = f(Wout).astype(np.float32)
    bout_f = f(bout).astype(np.float32)
    # [128 p, 8 cf, D] with cf = s*4+f mapping Wout rows s*512+f*128+p
    wout8 = np.ascontiguousarray(
        Wout_f.reshape(2, NQF, 128, D).transpose(2, 0, 1, 3).astype(BF16)
    ).reshape(128, 8, D)

    in_maps = []
    for c in range(NCORES):
        b, hh = c // 2, c % 2
        S = slice(hh * C, (hh + 1) * C)
        m = {}
        for s in STREAMS:
            m[f"xt_{s}"] = xts[b][s]
            m[f"wq_{s}"] = fold_slice(wq_full[s], S)
            m[f"wk_{s}"] = fold_slice(wk_full[s], S)
            m[f"bq_{s}"] = c32(bq_full[s][S])
            m[f"bk_{s}"] = c32(bk_full[s][S])
        m["wv"] = cbf(f(Wvv)[:, S])
        m["bv"] = cbf(f(bvv)[S]).reshape(1, C)
        m["wout8"] = wout8
        m["maskAB"] = maskABs[b]
        selv = np.zeros((128, 2, 512), dtype=np.float32)
        selv[:, hh, :] = 1.0
        m["sel"] = cbf(selv)
        m["ev_res"] = cbf(e_v[b, hh * 512 : (hh + 1) * 512, :] + bout_f[None, :])
        m["eye"] = cbf(np.eye(128, dtype=np.float32))
        m["gamma"] = c32(f(ln_gamma)).reshape(1, D)
        m["beta"] = c32(f(ln_beta)).reshape(1, D)
        in_maps.append(m)

    nc = _get_nc()
    trace = bool(os.environ.get("KERNEL_TRACE"))
    res = run_bass_kernel_spmd(
        nc, in_maps, core_ids=list(range(NCORES)), trace=trace
    )
    LAST_RESULT = res

    out_full = np.empty((B, L, D), dtype=np.float32)
    for c in range(NCORES):
        b, hh = c // 2, c % 2
        out_full[b, hh * 512 : (hh + 1) * 512, :] = res.results[c]["out"]
    return out_full


# revision 26
# speedup vs baseline: 2.1453x; 1.0850x over previous
"""DiffMHA (differential multi-head attention) block on 8 TRN2 NeuronCores.

Problem: B=4, L=1024, D=1024, H=16 heads (DH=64). Three input streams
(e_v, e_a0, e_a1); Q/K projections per stream, scores summed across
streams, causal-masked softmax, context from the v-stream values,
out-projection + residual + LayerNorm.

Sharding: (batch, head-half) -> 8 cores. Core c handles batch c//2 and
heads (c%2)*8 .. (c%2)*8+8. Each core computes its 8 heads' Q/K/V
projections (512 of 1024 channels), scores + softmax + context. Fold
context halves (128 channels x 512 rows) are exchanged between the two
cores of a batch via per-fold pairwise AllToAll DURING attention; each
core then runs the out-projection for its own 512 sequence rows with
the full 1024-channel contraction, then residual + LayerNorm locally.
No end-of-kernel collective.

Key optimizations over the v1 kernel:
- Causal skipping: score tiles with kt*128 > qb*256+255 are fully
  masked and skipped entirely (exp == 0 exactly); only diagonal-band
  tiles get the mask add. Attention matmul work drops ~40%.
- Stream packing: q/k of streams v and a0 are packed onto 128
  partitions (64 chans each) at projection-eviction time (partition-
  shifted PSUM->SBUF copies), so their two 64-deep score matmuls fuse
  into one 128-deep matmul; stream a1 stays a 64-deep accumulate.
- PSUM bank alternation: consecutive PE matmuls never accumulate into
  the same PSUM bank back-to-back (V-proj lf pairs, per-head score
  tiles, ctx of 2 heads, out-proj chains are interleaved), which keeps
  the PE at its ~216ns/512-col pipelined rate instead of ~430ns.
- Softmax normalization deferred past ctx accumulation via the extra
  ones-row of V (unchanged), reciprocal+broadcast per (head, q-half).
- DMA order: xt_v + wv first so the PE starts ~10us in, not ~46us.
"""

import os
import sys
import types

import ml_dtypes
import numpy as np

B, L, D, H = 4, 1024, 1024, 16
DH = D // H
HPC = H // 2  # heads per core
C = HPC * DH  # channels per core (512)
SCALE = float(1.0 / np.sqrt(DH))
EPS = 1e-12
NCORES = 8
BF16 = ml_dtypes.bfloat16


def _install_ntff_hook():
    """Recreate antenv.axon_hooks (absent in this image) so
    run_bass_kernel_spmd(trace=True) can capture NTFF profiles."""
    if "antenv.axon_hooks" in sys.modules:
        return
    try:
        from trn_agent_boot.trn_boot import _ntff_profile_via_ctypes

        hook = _ntff_profile_via_ctypes("/opt/axon/libaxon_pjrt.so")
    except Exception:
        hook = None
    mod = types.ModuleType("antenv.axon_hooks")
    mod.get_axon_ntff_profile_hook = lambda: hook
    mod.set_axon_ntff_profile_hook = lambda h: None
    sys.modules["antenv.axon_hooks"] = mod


_install_ntff_hook()

import concourse.bass as bass  # noqa: E402
import concourse.mybir as mybir  # noqa: E402
import concourse.tile as tile  # noqa: E402
from concourse import bacc  # noqa: E402
from concourse.bass_utils import run_bass_kernel_spmd  # noqa: E402

F32 = mybir.dt.float32
BF = mybir.dt.bfloat16
AF = mybir.ActivationFunctionType
ALU = mybir.AluOpType

_NC_CACHE = {}
LAST_RESULT = None

NQF = C // 128  # 4 channel folds per stream (2 heads each)
NLT = L // 128  # 8 l-tiles
NDT = D // 128  # 8 d-tiles (contraction)
NKT = L // 128  # 8 k-tiles
NRF = (L // 2) // 128  # 4 row tiles for out-proj/LN
STREAMS = ("v", "a0", "a1")
GROUPS = [[0, 1], [2, 3], [4, 5], [6, 7]]


def build_nc():
    nc = bacc.Bacc("TRN2", target_bir_lowering=False, debug=False, num_devices=NCORES)

    # ---- DRAM parameters (per-core shards, host-prepped) ----
    xt = {s: nc.declare_dram_parameter(f"xt_{s}", [D, L], BF, isOutput=False) for s in STREAMS}
    wq = {s: nc.declare_dram_parameter(f"wq_{s}", [NQF, D, 128], BF, isOutput=False) for s in STREAMS}
    wk = {s: nc.declare_dram_parameter(f"wk_{s}", [NQF, D, 128], BF, isOutput=False) for s in STREAMS}
    wv = nc.declare_dram_parameter("wv", [D, C], BF, isOutput=False)
    wout8 = nc.declare_dram_parameter("wout8", [128, 8, D], BF, isOutput=False)
    bq = {s: nc.declare_dram_parameter(f"bq_{s}", [C], F32, isOutput=False) for s in STREAMS}
    bk = {s: nc.declare_dram_parameter(f"bk_{s}", [C], F32, isOutput=False) for s in STREAMS}
    bv = nc.declare_dram_parameter("bv", [1, C], BF, isOutput=False)
    maskAB = nc.declare_dram_parameter("maskAB", [128, 2, 256], BF, isOutput=False)
    sel = nc.declare_dram_parameter("sel", [128, 2, 512], BF, isOutput=False)
    ev_res = nc.declare_dram_parameter("ev_res", [L // 2, D], BF, isOutput=False)
    eye = nc.declare_dram_parameter("eye", [128, 128], BF, isOutput=False)
    gamma = nc.declare_dram_parameter("gamma", [1, D], F32, isOutput=False)
    beta = nc.declare_dram_parameter("beta", [1, D], F32, isOutput=False)
    out = nc.declare_dram_parameter("out", [L // 2, D], F32, isOutput=True)


    with tile.TileContext(nc) as tc:
        with (
            tc.tile_pool(name="persist", bufs=1) as persist,
            tc.tile_pool(name="xtp", bufs=1) as xtp,
            tc.tile_pool(name="wf", bufs=10) as wf,
            tc.tile_pool(name="qkf", bufs=2) as qkf,
            tc.tile_pool(name="small", bufs=4) as small,
            tc.tile_pool(name="attn", bufs=4) as attn_pool,
            tc.tile_pool(name="ln", bufs=4) as ln_pool,
            tc.tile_pool(name="evp", bufs=4) as evp,
            tc.tile_pool(name="ctxf", bufs=2) as ctxf_pool,
            tc.tile_pool(name="proj_ps", bufs=3, space="PSUM") as proj_ps,
            tc.tile_pool(name="sc_ps", bufs=3, space="PSUM") as sc_ps,
            tc.tile_pool(name="ctx_ps", bufs=2, space="PSUM") as ctx_ps,
            tc.tile_pool(name="dram", bufs=1, space="DRAM") as dram,
        ):
            # ---- persistent SBUF tensors ----
            vnat = persist.tile([128, NLT, HPC, DH + 1], BF, tag="vnat")
            ctx_all = persist.tile([128, 8, L // 2], BF, tag="ctxall")
            mask_sb = persist.tile([128, 2, 256], BF, tag="maskAB")
            ones_b = persist.tile([1, L], BF, tag="ones")
            gb_bc = persist.tile([128, 2, D], F32, tag="gbbc")
            bv_sb = persist.tile([1, C], BF, tag="bvsb")
            wv_sb = persist.tile([128, NDT, C], BF, tag="wvsb")
            wout_sb = persist.tile([128, 8, D], BF, tag="woutsb")
            eps_sb = persist.tile([128, 1], F32, tag="eps")
            eye_sb = persist.tile([128, 128], BF, tag="eye")
            bq_sb = {
                s: persist.tile([128, NQF], F32, tag=f"bq{s}", name=f"bq_sb_{s}")
                for s in STREAMS
            }
            bk_sb = {
                s: persist.tile([128, NQF], F32, tag=f"bk{s}", name=f"bk_sb_{s}")
                for s in STREAMS
            }

            # ---- fold weight loader (lazy, cached) ----
            wf_cache = {}

            def load_wf(s, ff):
                if (s, ff) in wf_cache:
                    return wf_cache[(s, ff)]
                wq_t = wf.tile([128, NDT, 128], BF, tag="w", name=f"wq_{s}{ff}")
                wk_t = wf.tile([128, NDT, 128], BF, tag="w", name=f"wk_{s}{ff}")
                nc.sync.dma_start(
                    out=wq_t[:, :, :],
                    in_=wq[s][ff, :, :].rearrange("(dt p) c -> p dt c", p=128),
                )
                nc.sync.dma_start(
                    out=wk_t[:, :, :],
                    in_=wk[s][ff, :, :].rearrange("(dt p) c -> p dt c", p=128),
                )
                wf_cache[(s, ff)] = (wq_t, wk_t)
                return wq_t, wk_t

            # ---- preamble DMAs, in critical-path order: V-proj deps
            #      first, then fold-0 Q/K weights interleaved with the
            #      remaining embeddings; big late-use tensors (wout, ev,
            #      gamma/beta) are deferred into the fold loop. ----
            xt_sb = {}
            t = xtp.tile([128, NDT, L], BF, tag="xtv", name="xt_sb_v")
            nc.sync.dma_start(
                out=t[:, :, :], in_=xt["v"][:, :].rearrange("(dt p) l -> p dt l", p=128)
            )
            xt_sb["v"] = t
            nc.sync.dma_start(
                out=wv_sb[:, :, :], in_=wv[:, :].rearrange("(dt p) c -> p dt c", p=128)
            )
            nc.sync.dma_start(out=bv_sb[:, :], in_=bv[:, :])
            load_wf("v", 0)
            for s in ("a0", "a1"):
                t = xtp.tile([128, NDT, L], BF, tag=f"xt{s}", name=f"xt_sb_{s}")
                nc.sync.dma_start(
                    out=t[:, :, :],
                    in_=xt[s][:, :].rearrange("(dt p) l -> p dt l", p=128),
                )
                xt_sb[s] = t
                load_wf(s, 0)

            nc.vector.memset(ones_b[:, :], 1.0)
            nc.vector.memset(eps_sb[:, :], EPS)
            nc.vector.memset(vnat[:, :, :, DH : DH + 1], 1.0)

            nc.sync.dma_start(out=mask_sb[:, :, :], in_=maskAB[:, :, :])
            sel_sb = persist.tile([128, 2, 512], BF, tag="sel")
            nc.sync.dma_start(out=sel_sb[:, :, :], in_=sel[:, :, :])
            nc.sync.dma_start(out=eye_sb[:, :], in_=eye[:, :])
            for s in STREAMS:
                nc.sync.dma_start(
                    out=bq_sb[s][:, :], in_=bq[s][:].rearrange("(f p) -> p f", p=128)
                )
                nc.sync.dma_start(
                    out=bk_sb[s][:, :], in_=bk[s][:].rearrange("(f p) -> p f", p=128)
                )
            ev_sb = []

            # ---- V projection: natural [l, c] layout + ones column.
            #      lf pairs interleaved so consecutive matmuls alternate
            #      PSUM banks. ----
            for pair in range(NLT // 2):
                lf0, lf1 = 2 * pair, 2 * pair + 1
                psA = sc_ps.tile([128, C], F32, tag="sc")
                psB = sc_ps.tile([128, C], F32, tag="sc")
                for dt in range(NDT):
                    for lf, ps in ((lf0, psA), (lf1, psB)):
                        nc.tensor.matmul(
                            ps[:, :],
                            xt_sb["v"][:, dt, lf * 128 : (lf + 1) * 128],
                            wv_sb[:, dt, :],
                            start=(dt == 0),
                            stop=False,
                        )
                for lf, ps in ((lf0, psA), (lf1, psB)):
                    nc.tensor.matmul(
                        ps[:, :],
                        ones_b[:, lf * 128 : (lf + 1) * 128],
                        bv_sb[:, :],
                        start=False,
                        stop=True,
                    )
                nc.scalar.copy(vnat[:, lf0, :, 0:DH], psA[:, :])
                nc.scalar.copy(vnat[:, lf1, :, 0:DH], psB[:, :])

            # ---- fold-major main loop. Fold f+1's projections are
            #      emitted BEFORE fold f's attention so the packed-Q/K
            #      eviction latency hides under attention compute. ----
            fold_tiles = {}

            def emit_proj(f):
                # packed tiles: partitions [0:64] = stream v chans of the
                # head, [64:128] = stream a0 chans; a1 keeps fold layout.
                qpk = [
                    qkf.tile([128, L], BF, tag=f"qpk{hh}", name=f"qpk{hh}_{f}")
                    for hh in range(2)
                ]
                kpk = [
                    qkf.tile([128, L], BF, tag=f"kpk{hh}", name=f"kpk{hh}_{f}")
                    for hh in range(2)
                ]
                qa1 = qkf.tile([128, L], BF, tag="qa1", name=f"qa1_{f}")
                ka1 = qkf.tile([128, L], BF, tag="ka1", name=f"ka1_{f}")
                for s in STREAMS:
                    wq_t, wk_t = load_wf(s, f)
                    for w_t, b_t, pk, a1t in (
                        (wq_t, bq_sb[s], qpk, qa1),
                        (wk_t, bk_sb[s], kpk, ka1),
                    ):
                        ps = [
                            proj_ps.tile([128, 512], F32, tag="proj", name=f"pp{lh}")
                            for lh in range(2)
                        ]
                        for dt in range(NDT):
                            for lh in range(2):
                                nc.tensor.matmul(
                                    ps[lh][:, :],
                                    w_t[:, dt, :],
                                    xt_sb[s][:, dt, lh * 512 : (lh + 1) * 512],
                                    start=(dt == 0),
                                    stop=(dt == NDT - 1),
                                )
                        for lh in range(2):
                            lsl = slice(lh * 512, (lh + 1) * 512)
                            if s == "a1":
                                nc.scalar.activation(
                                    a1t[:, lsl], ps[lh][:, :], AF.Identity,
                                    bias=b_t[:, f : f + 1],
                                )
                            else:
                                off = 0 if s == "v" else 64
                                for hh in range(2):
                                    nc.scalar.activation(
                                        pk[hh][off : off + 64, lsl],
                                        ps[lh][hh * 64 : hh * 64 + 64, :],
                                        AF.Identity,
                                        bias=b_t[hh * 64 : hh * 64 + 64, f : f + 1],
                                    )
                fold_tiles[f] = (qpk, kpk, qa1, ka1)

            def emit_attention(f):
                qpk, kpk, qa1, ka1 = fold_tiles.pop(f)

                # stage late-use loads here so they don't compete with the
                # critical-path preamble/projection DMAs
                if f == 0:
                    nc.sync.dma_start(out=wout_sb[:, :, :], in_=wout8[:, :, :])
                if f == 1:
                    for rf in range(NRF):
                        t = evp.tile([128, D], BF, tag="ev", name=f"ev{rf}")
                        nc.sync.dma_start(
                            out=t[:, :], in_=ev_res[rf * 128 : (rf + 1) * 128, :]
                        )
                        ev_sb.append(t)
                if f == 2:
                    gsb = small.tile([1, D], F32, tag="gsb", bufs=1)
                    bsb = small.tile([1, D], F32, tag="bsb", bufs=1)
                    nc.sync.dma_start(out=gsb[:, :], in_=gamma[:, :])
                    nc.sync.dma_start(out=bsb[:, :], in_=beta[:, :])
                    nc.gpsimd.partition_broadcast(gb_bc[:, 0, :], gsb[:, :])
                    nc.gpsimd.partition_broadcast(gb_bc[:, 1, :], bsb[:, :])

                ctxf = ctxf_pool.tile([128, L], BF, tag="ctxf", name=f"ctxf{f}")
                cxs = ctxf_pool.tile(
                    [128, 2, 2, 512], BF, tag="cxs", name=f"cxs{f}", bufs=1
                )
                cx_in = dram.tile(
                    [2, 2, 128, 512], BF, tag=f"cxin{f}", name=f"cxin{f}"
                )
                for qh in range(2):
                    cps = [
                        ctx_ps.tile([DH + 1, 512], F32, tag="ctx", name=f"cps{i}")
                        for i in range(2)
                    ]
                    n_kt = 4 * qh + 4  # live k-tiles for this q-half
                    sps_at = {}

                    def emit_scores(kt):
                        sps = [
                            sc_ps.tile([128, 512], F32, tag="sc", name=f"sps{i}")
                            for i in range(2)
                        ]
                        # PE issue is ~216ns/instr regardless of width, so
                        # use one full 512-col matmul pair when both q
                        # halves are live; 256-col only on the causal edge.
                        full = kt <= 4 * qh + 1
                        qsl = (
                            slice(qh * 512, qh * 512 + 512)
                            if full
                            else slice(qh * 512 + 256, qh * 512 + 512)
                        )
                        osl = slice(0, 512) if full else slice(256, 512)
                        ksl = slice(kt * 128, (kt + 1) * 128)
                        for hh in range(2):
                            nc.tensor.matmul(
                                sps[hh][:, osl],
                                kpk[hh][:, ksl],
                                qpk[hh][:, qsl],
                                start=True,
                                stop=False,
                            )
                        for hh in range(2):
                            p0 = hh * 64
                            nc.tensor.matmul(
                                sps[hh][:, osl],
                                ka1[p0 : p0 + 64, ksl],
                                qa1[p0 : p0 + 64, qsl],
                                start=False,
                                stop=True,
                            )
                        # mask only on diagonal-band halves
                        for qbh in range(2):
                            qb = 2 * qh + qbh
                            if kt in (2 * qb, 2 * qb + 1):
                                msl = slice(qbh * 256, qbh * 256 + 256)
                                for hh in range(2):
                                    nc.vector.tensor_add(
                                        sps[hh][:, msl],
                                        sps[hh][:, msl],
                                        mask_sb[:, kt % 2, :],
                                    )
                        # exp -> bf16 attn tiles (dead qb0 half zeroed so
                        # the full-width ctx matmul accumulates one group
                        # per PSUM bank)
                        at = [
                            attn_pool.tile([128, 512], BF, tag="attn", name=f"at{i}")
                            for i in range(2)
                        ]
                        for hh in range(2):
                            if not full:
                                nc.vector.memset(at[hh][:, 0:256], 0.0)
                            nc.scalar.activation(
                                at[hh][:, osl], sps[hh][:, osl], AF.Exp, scale=SCALE
                            )
                        sps_at[kt] = at

                    def emit_ctx(kt):
                        at = sps_at.pop(kt)
                        for hh in range(2):
                            h = 2 * f + hh
                            nc.tensor.matmul(
                                cps[hh][:, :],
                                vnat[:, kt, h, :],
                                at[hh][:, :],
                                start=(kt == 0),
                                stop=(kt == n_kt - 1),
                            )

                    prev = None
                    for kt in range(n_kt):
                        emit_scores(kt)
                        if prev is not None:
                            emit_ctx(prev)
                        prev = kt
                    emit_ctx(prev)

                    # normalize: divide ctx rows by the attn row-sums that
                    # accumulated in psum row DH (sum staged to SBUF for the
                    # fast custom-DVE reciprocal, which is SBUF-only)
                    for hh in range(2):
                        p0 = hh * 64
                        sr = small.tile([1, 512], F32, tag="sr", bufs=2)
                        nc.scalar.copy(sr[:, :], cps[hh][DH : DH + 1, :])
                        inv = small.tile([1, 512], F32, tag="inv", bufs=2)
                        nc.vector.reciprocal_approx_fast(inv[:, :], sr[:, :])
                        inv_bc = small.tile([64, 512], F32, tag="invbc", bufs=2)
                        nc.gpsimd.partition_broadcast(inv_bc[:, :], inv[:, :])
                        nc.vector.tensor_mul(
                            ctxf[p0 : p0 + 64, qh * 512 : (qh + 1) * 512],
                            cps[hh][0:DH, :],
                            inv_bc[:, :],
                        )

                    # stage this q-half (= dest-rank chunk) for the
                    # exchange as soon as it is normalized
                    for s2 in range(2):
                        nc.vector.tensor_mul(
                            cxs[:, qh, s2, :],
                            ctxf[:, qh * 512 : (qh + 1) * 512],
                            sel_sb[:, s2, :],
                        )
                        nc.sync.dma_start(
                            out=cx_in[qh, s2, :, :], in_=cxs[:, qh, s2, :]
                        )

                # -- exchange fold ctx halves with the pair core.
                # AllToAll isn't available for 2-core groups, so emulate it
                # with a ReduceScatter over [dest d][chan-slot s] staging
                # where slot s != own-half is zeroed via the host-provided
                # 0/1 `sel` tensor (x + 0 is exact in bf16). Rank d then
                # receives [ctx_half0, ctx_half1] for its own rows. --
                cx_out = dram.tile(
                    [2, 128, 512], BF, tag=f"cxout{f}", name=f"cxout{f}"
                )
                nc.gpsimd.collective_compute(
                    "ReduceScatter",
                    ALU.add,
                    replica_groups=GROUPS,
                    ins=[cx_in.opt()],
                    outs=[cx_out.opt()],
                )
                for s2 in range(2):
                    nc.sync.dma_start(
                        out=ctx_all[:, s2 * NQF + f, :], in_=cx_out[s2, :, :]
                    )

            emit_proj(0)
            for f in range(NQF):
                if f + 1 < NQF:
                    emit_proj(f + 1)
                emit_attention(f)

            # ---- out-projection over full 1024 channels for own rows ----
            # 8 chains (lt, dh). Chains for lt 0,1,3 are partially
            # accumulated (folds 0-2 contributions + residual) right after
            # fold-3 attention, filling the PE idle window while fold 3's
            # normalize/exchange runs; the fold-3 contributions and the lt2
            # chains run after the last readback.
            early_chains = [(lt, dh) for lt in (0, 1, 3) for dh in range(2)]
            late_chains = [(2, 0), (2, 1)]
            pools = [sc_ps, proj_ps]
            ptags = ["sc", "proj"]
            ops = {}
            for i, ch in enumerate(early_chains):
                ops[ch] = pools[i % 2].tile(
                    [128, 512], F32, tag=ptags[i % 2], name=f"opse{i}"
                )

            def op_mm(ch, cf, start, stop):
                lt, dh = ch
                nc.tensor.matmul(
                    ops[ch][:, :],
                    ctx_all[:, cf, lt * 128 : (lt + 1) * 128],
                    wout_sb[:, cf, dh * 512 : (dh + 1) * 512],
                    start=start,
                    stop=stop,
                )

            def op_eye(ch, stop):
                lt, dh = ch
                nc.tensor.matmul(
                    ops[ch][:, :],
                    eye_sb[:, :],
                    ev_sb[lt][:, dh * 512 : (dh + 1) * 512],
                    start=False,
                    stop=stop,
                )

            for cf in (0, 1, 2, 4, 5, 6):
                for ch in early_chains:
                    op_mm(ch, cf, start=(cf == 0), stop=False)
            for ch in early_chains:
                op_eye(ch, stop=False)
            # ---- late part: fold-3 contributions ----
            for cf in (3, 7):
                for ch in early_chains:
                    op_mm(ch, cf, start=False, stop=(cf == 7))
            for i, ch in enumerate(late_chains):
                ops[ch] = pools[i % 2].tile(
                    [128, 512], F32, tag=ptags[i % 2], name=f"opsl{i}"
                )
            for cf in range(8):
                for ch in late_chains:
                    op_mm(ch, cf, start=(cf == 0), stop=False)
            for ch in late_chains:
                op_eye(ch, stop=True)

            # ---- evict + LayerNorm per row tile ----
            for lt in (0, 1, 3, 2):
                lsl = slice(lt * 128, (lt + 1) * 128)
                xt_ = ln_pool.tile([128, D], F32, tag="x", name=f"x{lt}")
                nc.scalar.copy(xt_[:, 0:512], ops[(lt, 0)][:, :])
                nc.scalar.copy(xt_[:, 512:1024], ops[(lt, 1)][:, :])
                stats = small.tile([128, 2, 6], F32, tag="stats")
                nc.vector.bn_stats(out=stats[:, 0, :], in_=xt_[:, 0:512])
                nc.vector.bn_stats(out=stats[:, 1, :], in_=xt_[:, 512:1024])
                mv = small.tile([128, 2], F32, tag="mv")
                nc.vector.bn_aggr(out=mv[:, :], in_=stats[:, :, :])
                std = small.tile([128, 1], F32, tag="std")
                nc.scalar.activation(std[:, :], mv[:, 1:2], AF.Sqrt, bias=eps_sb[:, :])
                rstd = small.tile([128, 1], F32, tag="rstd")
                nc.vector.reciprocal(rstd[:, :], std[:, :])
                negmb = small.tile([128, 1], F32, tag="negmb")
                nc.vector.scalar_tensor_tensor(
                    negmb[:, :],
                    mv[:, 0:1],
                    -1.0,
                    rstd[:, :],
                    op0=ALU.mult,
                    op1=ALU.mult,
                )
                nc.scalar.activation(
                    xt_[:, :],
                    xt_[:, :],
                    AF.Identity,
                    bias=negmb[:, :],
                    scale=rstd[:, :],
                )
                nc.vector.tensor_mul(xt_[:, :], xt_[:, :], gb_bc[:, 0, :])
                nc.vector.tensor_add(xt_[:, :], xt_[:, :], gb_bc[:, 1, :])
                nc.sync.dma_start(out=out[lsl, :], in_=xt_[:, :])

    nc.compile()
    return nc


def _get_nc():
    if "nc" not in _NC_CACHE:
        _NC_CACHE["nc"] = build_nc()
    return _NC_CACHE["nc"]


def kernel(
    e_v, e_a0, e_a1, Wqv, bqv, Wkv, bkv, Wvv, bvv,
    Wqa0, bqa0, Wka0, bka0, Wqa1, bqa1, Wka1, bka1,
    Wout, bout, ln_gamma, ln_beta, attn_mask,
):
    global LAST_RESULT
    f = np.asarray
    e_v, e_a0, e_a1 = f(e_v), f(e_a0), f(e_a1)
    attn_mask = f(attn_mask)
    c32 = lambda a: np.ascontiguousarray(a, dtype=np.float32)
    cbf = lambda a: np.ascontiguousarray(np.asarray(a, dtype=np.float32).astype(BF16))

    wq_full = {"v": f(Wqv), "a0": f(Wqa0), "a1": f(Wqa1)}
    wk_full = {"v": f(Wkv), "a0": f(Wka0), "a1": f(Wka1)}
    bq_full = {"v": f(bqv), "a0": f(bqa0), "a1": f(bqa1)}
    bk_full = {"v": f(bkv), "a0": f(bka0), "a1": f(bka1)}

    xts = {}
    maskABs = {}
    for b in range(B):
        xts[b] = {
            "v": cbf(e_v[b].T),
            "a0": cbf(e_a0[b].T),
            "a1": cbf(e_a1[b].T),
        }
        mT = f(attn_mask[b, 0]).T * (1.0 / SCALE)
        # diagonal-band mask patterns: A = (kt == 2*qb), B = (kt == 2*qb+1)
        maskABs[b] = cbf(np.stack([mT[0:128, 0:256], mT[128:256, 0:256]], axis=1))

    def fold_slice(w, S):
        # [D, C] slice -> [NQF, D, 128] fold-major
        ws = np.asarray(w[:, S], dtype=np.float32)  # [D, C]
        return np.ascontiguousarray(
            ws.reshape(D, NQF, 128).transpose(1, 0, 2).astype(BF16)
        )

    Wout_f = f(Wout).astype(np.float32)
    bout_f = f(bout).astype(np.float32)
    # [128 p, 8 cf, D] with cf = s*4+f mapping Wout rows s*512+f*128+p
    wout8 = np.ascontiguousarray(
        Wout_f.reshape(2, NQF, 128, D).transpose(2, 0, 1, 3).astype(BF16)
    ).reshape(128, 8, D)

    in_maps = []
    for c in range(NCORES):
        b, hh = c // 2, c % 2
        S = slice(hh * C, (hh + 1) * C)
        m = {}
        for s in STREAMS:
            m[f"xt_{s}"] = xts[b][s]
            m[f"wq_{s}"] = fold_slice(wq_full[s], S)
            m[f"wk_{s}"] = fold_slice(wk_full[s], S)
            m[f"bq_{s}"] = c32(bq_full[s][S])
            m[f"bk_{s}"] = c32(bk_full[s][S])
        m["wv"] = cbf(f(Wvv)[:, S])
        m["bv"] = cbf(f(bvv)[S]).reshape(1, C)
        m["wout8"] = wout8
        m["maskAB"] = maskABs[b]
        selv = np.zeros((128, 2, 512), dtype=np.float32)
        selv[:, hh, :] = 1.0
        m["sel"] = cbf(selv)
        m["ev_res"] = cbf(e_v[b, hh * 512 : (hh + 1) * 512, :] + bout_f[None, :])
        m["eye"] = cbf(np.eye(128, dtype=np.float32))
        m["gamma"] = c32(f(ln_gamma)).reshape(1, D)
        m["beta"] = c32(f(ln_beta)).reshape(1, D)
        in_maps.append(m)

    nc = _get_nc()
    trace = bool(os.environ.get("KERNEL_TRACE"))
    res = run_bass_kernel_spmd(
        nc, in_maps, core_ids=list(range(NCORES)), trace=trace
    )
    LAST_RESULT = res

    out_full = np.empty((B, L, D), dtype=np.float32)
    for c in range(NCORES):
        b, hh = c // 2, c % 2
        out_full[b, hh * 512 : (hh + 1) * 512, :] = res.results[c]["out"]
    return out_full
